# revision 1
# baseline (speedup 1.0000x reference)
"""Trainium2 Bass kernel for nn_MetaLearner (meta-learning attention + cosine
prototype scoring), data-parallel over tasks on 8 NeuronCores.

Math (per task):
  c   = [img, txt] @ Wc.T + bc                (Wc = concat(Wi, Wt))
  h   = LN1(c);  q,k,v = h @ W{q,k,v}.T + b   (queries: seqlen=1 -> ctx = v)
  ctx = softmax(q k^T / sqrt(128)) v          (support: seqlen=4)
  f   = LN2(ctx) @ Wo.T + bo
  logits[t,q,c] = 10 * cos(qf[t,q], sf[t,c])

Host-side folds (all linear, exact):
  - LN gains/biases folded into the following projection weights.
  - LN mean-subtraction folded into the producing weights (column-centered
    weights make the projection output zero-mean; attention outputs of
    centered v stay centered since softmax rows sum to 1).
  - 1/sqrt(128) folded into Wq; the x10 cosine scale into the support norms.
  - Inputs are pre-transposed on host so the contraction dim lands on SBUF
    partitions (f32 DMA-transpose is not available on TRN2's xbar).
On-chip layout is "transposed" throughout: activations are [hid, rows].
Matmul operands use float32r (TF32-like, 1 cyc/row); vector/scalar compute
stays float32.
"""
import sys
sys.path.insert(0, "/opt/trn_rl_repo")
import numpy as np

HID = 128
T, Q, S = 256, 64, 4
DI, DTXT = 2048, 768
NCORES = 8
TPC = T // NCORES               # 32 tasks per core
FEAT = DI + DTXT                # 2816
KT = FEAT // 128                # 22 contraction chunks
QROWS = TPC * Q                 # 2048 query rows per core
SROWS = TPC * S                 # 128 support rows per core
ROWS = QROWS + SROWS            # 2176
CHUNKS = [(0, 512), (512, 512), (1024, 512), (1536, 512)]  # query chunks
SCALE_INV = 1.0 / (np.sqrt(HID) + 1e-8)
EPS = 1e-5

_prog = None  # cached compiled Bass program


def _build():
    import concourse.bacc as bacc
    import concourse.tile as tile
    import concourse.mybir as mybir
    import concourse.bass as _b

    F32 = mybir.dt.float32
    F32R = mybir.dt.float32r
    AFT = mybir.ActivationFunctionType
    AX = mybir.AxisListType
    ALU = mybir.AluOpType

    nc = bacc.Bacc()
    xT_d = nc.declare_dram_parameter("xT", [FEAT, ROWS], F32R, isOutput=False)
    wc_d = nc.declare_dram_parameter("wc", [FEAT, HID], F32R, isOutput=False)
    wqkvo_d = nc.declare_dram_parameter("wqkvo", [HID, 4 * HID], F32R,
                                        isOutput=False)
    bias_d = nc.declare_dram_parameter("biases", [HID, 6], F32, isOutput=False)
    onesr_d = nc.declare_dram_parameter("onesr", [1, HID], F32R, isOutput=False)
    mask_d = nc.declare_dram_parameter("mask", [SROWS, SROWS], F32, isOutput=False)
    id_d = nc.declare_dram_parameter("ident", [128, 128], F32, isOutput=False)
    out_d = nc.declare_dram_parameter("logits", [TPC, Q, S], F32, isOutput=True)

    lp = nc.allow_low_precision(reason="float32r tiles are bit-compatible f32")
    lp.__enter__()

    ACHUNKS = CHUNKS + [(QROWS, SROWS)]   # 4 query chunks + support chunk

    with tile.TileContext(nc) as tc:
        with (
            tc.tile_pool(name="wts", bufs=1) as wts,
            tc.tile_pool(name="qfp", bufs=1) as qfp,
            tc.tile_pool(name="xp", bufs=3) as xp,
            tc.tile_pool(name="wk", bufs=2) as wk,
            tc.tile_pool(name="ps", bufs=1, space="PSUM") as ps,
        ):
            # ---- stream-critical loads first (SP issues in program order) ----
            wc_t = wts.tile([128, KT, HID], F32R)
            nc.sync.dma_start(out=wc_t[:, 0, :], in_=wc_d[0:128, :])
            xa_tiles = []

            def _xa(k):
                x_t = xp.tile([128, 640], F32R, tag="xa", name=f"xa{k}")
                nc.sync.dma_start(out=x_t,
                                  in_=xT_d[k * 128:(k + 1) * 128, 0:640])
                xa_tiles.append(x_t)

            def _wc(k0, k1):
                nc.sync.dma_start(
                    out=wc_t[:, k0:k1, :],
                    in_=wc_d[k0 * 128:k1 * 128].rearrange(
                        "(c p) m -> p c m", p=128))

            _xa(0)
            _wc(1, 3)
            _xa(1)
            _wc(3, 8)
            _xa(2)
            _wc(8, KT)
            wqkvo_t = wts.tile([128, 4 * HID], F32R)
            nc.sync.dma_start(out=wqkvo_t, in_=wqkvo_d[:])
            wq_t = wqkvo_t[:, 0 * HID:1 * HID]
            wk_t = wqkvo_t[:, 1 * HID:2 * HID]
            wv_t = wqkvo_t[:, 2 * HID:3 * HID]
            wo_t = wqkvo_t[:, 3 * HID:4 * HID]
            bias_t = wts.tile([HID, 6], F32)
            nc.sync.dma_start(out=bias_t, in_=bias_d[:])
            bc_t = bias_t[:, 0:1]
            bq_t = bias_t[:, 1:2]
            bk_t = bias_t[:, 2:3]
            bv_t = bias_t[:, 3:4]
            bo_t = bias_t[:, 4:5]
            ones_c = wts.tile([128, 1], F32R)    # ss-reduce lhsT [K=128, M=1]
            nc.gpsimd.dma_start(out=ones_c, in_=bias_d[:, 5:6])
            ones_r = wts.tile([1, 128], F32R)    # broadcast lhsT [K=1, M=128]
            nc.sync.dma_start(out=ones_r, in_=onesr_d[:])
            eps_t = wts.tile([1, 1], F32)
            nc.vector.memset(eps_t, EPS)

            qf_tiles = [qfp.tile([128, 512], F32R, tag=f"qf{n}", name=f"qf{n}")
                        for n in range(4)]
            sf_t = qfp.tile([128, SROWS], F32R, tag="sft")

            def rstd_bcast_sb(psum, src_sb, cn, scale):
                """SBUF f32 src -> 1/sqrt(mean(src^2)+eps) bcast [128,cn] PSUM."""
                sq = wk.tile([128, 512], F32R, tag="sq")
                nc.scalar.activation(out=sq[:, :cn], in_=src_sb[:, :cn],
                                     func=AFT.Square, bias=0.0, scale=1.0)
                ss_ps = psum.tile([1, 512], F32, tag="ssps", bufs=2)
                nc.tensor.matmul(ss_ps[:, :cn], ones_c[:], sq[:, :cn],
                                 start=True, stop=True)
                var_r = wk.tile([1, 512], F32, tag="var")
                nc.scalar.activation(out=var_r[:, :cn], in_=ss_ps[:, :cn],
                                     func=AFT.Sqrt, bias=eps_t[:], scale=scale)
                rstd_r = wk.tile([1, 512], F32R, tag="rstdr")
                nc.vector.reciprocal(out=rstd_r[:, :cn], in_=var_r[:, :cn])
                R_ps = psum.tile([128, 512], F32, tag="rps", bufs=2)
                nc.tensor.matmul(R_ps[:, :cn], ones_r[:], rstd_r[:, :cn],
                                 start=True, stop=True)
                return R_ps

            def rstd_bcast_ps(psum, src_ps, bias, cn, scale):
                """PSUM src (+bias) -> rstd bcast [128,cn] PSUM."""
                sq = wk.tile([128, 512], F32R, tag="sq2")
                nc.scalar.activation(out=sq[:, :cn], in_=src_ps[:, :cn],
                                     func=AFT.Square, bias=bias, scale=1.0)
                ss_ps = psum.tile([1, 512], F32, tag="ssps", bufs=2)
                nc.tensor.matmul(ss_ps[:, :cn], ones_c[:], sq[:, :cn],
                                 start=True, stop=True)
                var_r = wk.tile([1, 512], F32, tag="var")
                nc.scalar.activation(out=var_r[:, :cn], in_=ss_ps[:, :cn],
                                     func=AFT.Sqrt, bias=eps_t[:], scale=scale)
                rstd_r = wk.tile([1, 512], F32R, tag="rstdr")
                nc.vector.reciprocal(out=rstd_r[:, :cn], in_=var_r[:, :cn])
                R_ps = psum.tile([128, 512], F32, tag="rps", bufs=2)
                nc.tensor.matmul(R_ps[:, :cn], ones_r[:], rstd_r[:, :cn],
                                 start=True, stop=True)
                return R_ps

            def col_normalize(psum, dst, src_ps, bias, cn, sqs, clip):
                """dst = (src+bias) / max(||col||*sqs, clip) per column."""
                sq = wk.tile([128, 512], F32R, tag="sq3")
                nc.scalar.activation(out=sq[:, :cn], in_=src_ps[:, :cn],
                                     func=AFT.Square, bias=bias, scale=1.0)
                ss_ps = psum.tile([1, 512], F32, tag="ssps", bufs=2)
                nc.tensor.matmul(ss_ps[:, :cn], ones_c[:], sq[:, :cn],
                                 start=True, stop=True)
                n_r = wk.tile([1, 512], F32, tag="nrm")
                nc.scalar.activation(out=n_r[:, :cn], in_=ss_ps[:, :cn],
                                     func=AFT.Sqrt, bias=0.0, scale=sqs)
                nc.vector.tensor_scalar_max(out=n_r[:, :cn], in0=n_r[:, :cn],
                                            scalar1=clip)
                i_r = wk.tile([1, 512], F32R, tag="inrm")
                nc.vector.reciprocal(out=i_r[:, :cn], in_=n_r[:, :cn])
                I_ps = psum.tile([128, 512], F32, tag="rps", bufs=2)
                nc.tensor.matmul(I_ps[:, :cn], ones_r[:], i_r[:, :cn],
                                 start=True, stop=True)
                raw = wk.tile([128, 512], F32, tag="qraw")
                nc.scalar.activation(out=raw[:, :cn], in_=src_ps[:, :cn],
                                     func=AFT.Identity, bias=bias, scale=1.0)
                nc.vector.tensor_mul(out=dst[:, :cn], in0=raw[:, :cn],
                                     in1=I_ps[:, :cn])

            def query_tail(pst, qf_tile, qf_off, c_f, cn):
                R1 = rstd_bcast_sb(pst, c_f, cn, 1.0 / HID)
                h_t = wk.tile([128, 512], F32R, tag="h")
                nc.vector.tensor_mul(out=h_t[:, :cn], in0=c_f[:, :cn],
                                     in1=R1[:, :cn])
                v_ps = pst.tile([128, 512], F32, tag="pps", bufs=2)
                nc.tensor.matmul(v_ps[:, :cn], wv_t, h_t[:, :cn],
                                 start=True, stop=True)
                R2 = rstd_bcast_ps(pst, v_ps, bv_t, cn, 1.0 / HID)
                v_f = wk.tile([128, 512], F32, tag="vf")
                nc.scalar.activation(out=v_f[:, :cn], in_=v_ps[:, :cn],
                                     func=AFT.Identity, bias=bv_t, scale=1.0)
                z_t = wk.tile([128, 512], F32R, tag="z")
                nc.vector.tensor_mul(out=z_t[:, :cn], in0=v_f[:, :cn],
                                     in1=R2[:, :cn])
                o_ps = pst.tile([128, 512], F32, tag="pps", bufs=2)
                nc.tensor.matmul(o_ps[:, :cn], wo_t, z_t[:, :cn],
                                 start=True, stop=True)
                dst = qf_tile[:, qf_off:qf_off + cn]
                col_normalize(pst, dst, o_ps, bo_t, cn, 1.0, 1e-8)

            # streaming groups, ascending completion time; every tail except
            # the last group's hides under a later group's DMA stream
            # (xT col layout: [support | q0 | q1 | q2 | q3])
            G1 = [("s", 0, SROWS), (0, SROWS, 512)]
            G2 = [(1, 640, 512), (2, 1152, 512)]
            G3 = [("3a", 1664, 256), ("3b", 1920, 256)]
            c_fs = {}

            with tc.tile_pool(name="pst", bufs=1, space="PSUM") as pst:
                # ---- group 1 ----
                with tc.tile_pool(name="ps1", bufs=1, space="PSUM") as ps1:
                    c_pss = {n: ps1.tile([128, cn], F32, tag=f"cps{n}",
                                         name=f"cps_{n}")
                             for (n, c0, cn) in G1}
                    for k in range(KT):
                        if k < 3:
                            x_t = xa_tiles[k]
                        else:
                            x_t = xp.tile([128, 640], F32R, tag="xa")
                            nc.sync.dma_start(
                                out=x_t, in_=xT_d[k * 128:(k + 1) * 128, 0:640])
                        for (n, c0, cn) in G1:
                            nc.tensor.matmul(c_pss[n][:, :cn], wc_t[:, k, :],
                                             x_t[:, c0:c0 + cn],
                                             start=(k == 0), stop=(k == KT - 1))
                    for (n, c0, cn) in G1:
                        c_f = wk.tile([128, 512], F32, tag=f"cf{n}",
                                      name=f"cf{n}")
                        nc.scalar.activation(out=c_f[:, :cn],
                                             in_=c_pss[n][:, :cn],
                                             func=AFT.Identity, bias=bc_t,
                                             scale=1.0)
                        c_fs[n] = c_f

                mask_t = wts.tile([SROWS, SROWS], F32)
                nc.sync.dma_start(out=mask_t, in_=mask_d[:])
                id_t = wts.tile([128, 128], F32)
                nc.sync.dma_start(out=id_t, in_=id_d[:])

                # ---- group 2 streams; support + q0 tails run underneath ----
                with tc.tile_pool(name="ps2", bufs=1, space="PSUM") as ps2:
                    c_pss = {n: ps2.tile([128, cn], F32, tag=f"cps{n}",
                                         name=f"cps_{n}")
                             for (n, c0, cn) in G2}
                    for k in range(KT):
                        x_t = xp.tile([128, 1024], F32R, tag="xb")
                        nc.sync.dma_start(
                            out=x_t, in_=xT_d[k * 128:(k + 1) * 128, 640:1664])
                        for (n, c0, cn) in G2:
                            nc.tensor.matmul(c_pss[n][:, :cn], wc_t[:, k, :],
                                             x_t[:, c0 - 640:c0 - 640 + cn],
                                             start=(k == 0), stop=(k == KT - 1))

                    # support tail
                    cn = SROWS
                    cs_f = c_fs["s"]
                    R1 = rstd_bcast_sb(pst, cs_f, cn, 1.0 / HID)
                    h_t = wk.tile([128, SROWS], F32R, tag="sh")
                    nc.vector.tensor_mul(out=h_t, in0=cs_f[:, :cn],
                                         in1=R1[:, :cn])
                    q_ps = pst.tile([128, SROWS], F32, tag="pps", bufs=2)
                    nc.tensor.matmul(q_ps[:], wq_t, h_t[:], start=True, stop=True)
                    qT = wk.tile([128, SROWS], F32R, tag="qT")
                    nc.scalar.activation(out=qT, in_=q_ps, func=AFT.Identity,
                                         bias=bq_t, scale=1.0)
                    k_ps = pst.tile([128, SROWS], F32, tag="pps", bufs=2)
                    nc.tensor.matmul(k_ps[:], wk_t, h_t[:], start=True, stop=True)
                    kT = wk.tile([128, SROWS], F32R, tag="kT")
                    nc.scalar.activation(out=kT, in_=k_ps, func=AFT.Identity,
                                         bias=bk_t, scale=1.0)
                    v_ps = pst.tile([128, SROWS], F32, tag="pps", bufs=2)
                    nc.tensor.matmul(v_ps[:], wv_t, h_t[:], start=True, stop=True)
                    vT_f = wk.tile([128, SROWS], F32, tag="vTf")
                    nc.scalar.activation(out=vT_f, in_=v_ps, func=AFT.Identity,
                                         bias=bv_t, scale=1.0)

                    s_ps = pst.tile([SROWS, SROWS], F32, tag="rps", bufs=2)
                    nc.tensor.matmul(s_ps[:], qT[:], kT[:], start=True, stop=True)
                    s_f = wk.tile([SROWS, SROWS], F32, tag="sf_")
                    nc.vector.tensor_add(out=s_f, in0=s_ps, in1=mask_t)
                    nmx = wk.tile([SROWS, 1], F32, tag="nmx")
                    nc.vector.tensor_reduce(out=nmx, in_=s_f, axis=AX.X,
                                            op=ALU.max, negate=True)
                    a_f = wk.tile([SROWS, SROWS], F32, tag="af")
                    asum = wk.tile([SROWS, 1], F32, tag="asum")
                    nc.scalar.activation(out=a_f, in_=s_f, func=AFT.Exp,
                                         bias=nmx, scale=1.0, accum_out=asum)
                    rs = wk.tile([SROWS, 1], F32, tag="rs")
                    nc.vector.reciprocal(out=rs, in_=asum)
                    nc.vector.tensor_scalar_mul(out=a_f, in0=a_f, scalar1=rs)

                    aT_ps = pst.tile([SROWS, SROWS], F32, tag="pps", bufs=2)
                    nc.tensor.matmul(aT_ps[:], a_f[:], id_t[:], is_transpose=True)
                    aT = wk.tile([SROWS, SROWS], F32, tag="aT")
                    nc.vector.tensor_copy(out=aT, in_=aT_ps)
                    vn_ps = pst.tile([SROWS, SROWS], F32, tag="pps", bufs=2)
                    nc.tensor.matmul(vn_ps[:], vT_f[:], id_t[:], is_transpose=True)
                    vn = wk.tile([SROWS, SROWS], F32, tag="vn")
                    nc.vector.tensor_copy(out=vn, in_=vn_ps)
                    ctx_ps = pst.tile([128, SROWS], F32, tag="pps", bufs=2)
                    nc.tensor.matmul(ctx_ps[:], vn[:], aT[:], start=True, stop=True)
                    ctx_f = wk.tile([128, SROWS], F32, tag="ctxf")
                    nc.scalar.activation(out=ctx_f, in_=ctx_ps, func=AFT.Copy,
                                         scale=1.0)

                    R2 = rstd_bcast_sb(pst, ctx_f, cn, 1.0 / HID)
                    z_t = wk.tile([128, SROWS], F32R, tag="sz")
                    nc.vector.tensor_mul(out=z_t, in0=ctx_f, in1=R2[:, :cn])
                    o_ps = pst.tile([128, SROWS], F32, tag="pps", bufs=2)
                    nc.tensor.matmul(o_ps[:], wo_t, z_t[:], start=True, stop=True)
                    col_normalize(pst, sf_t, o_ps, bo_t, cn, 0.01, 1e-9)

                    # q0 tail
                    query_tail(pst, qf_tiles[0], 0, c_fs[0], 512)

                    for (n, c0, cn) in G2:
                        c_f = wk.tile([128, 512], F32, tag=f"cf{n}",
                                      name=f"cf{n}")
                        nc.scalar.activation(out=c_f[:, :cn],
                                             in_=c_pss[n][:, :cn],
                                             func=AFT.Identity, bias=bc_t,
                                             scale=1.0)
                        c_fs[n] = c_f

                # ---- group 3 streams; q1/q2 tails underneath ----
                with tc.tile_pool(name="ps3", bufs=1, space="PSUM") as ps3:
                    c_pss = {n: ps3.tile([128, cn], F32, tag=f"cps{n}",
                                         name=f"cps_{n}")
                             for (n, c0, cn) in G3}
                    for k in range(KT):
                        x_t = xp.tile([128, 512], F32R, tag="xc")
                        nc.sync.dma_start(
                            out=x_t, in_=xT_d[k * 128:(k + 1) * 128, 1664:ROWS])
                        for (n, c0, cn) in G3:
                            nc.tensor.matmul(c_pss[n][:, :cn], wc_t[:, k, :],
                                             x_t[:, c0 - 1664:c0 - 1664 + cn],
                                             start=(k == 0), stop=(k == KT - 1))

                    query_tail(pst, qf_tiles[1], 0, c_fs[1], 512)
                    query_tail(pst, qf_tiles[2], 0, c_fs[2], 512)

                    for (n, c0, cn) in G3:
                        c_f = wk.tile([128, 512], F32, tag=f"cf{n}",
                                      name=f"cf{n}")
                        nc.scalar.activation(out=c_f[:, :cn],
                                             in_=c_pss[n][:, :cn],
                                             func=AFT.Identity, bias=bc_t,
                                             scale=1.0)
                        c_fs[n] = c_f

                # ---- last tails (half-width, short chains) ----
                query_tail(pst, qf_tiles[3], 0, c_fs["3a"], 256)
                query_tail(pst, qf_tiles[3], 256, c_fs["3b"], 256)

                # ---- per-task scores + output ----
                with tc.tile_pool(name="psu", bufs=1, space="PSUM") as psu:
                    U_ps = psu.tile([64, 2 * Q], F32, tag="ups", name="U_ps")
                    for t in range(TPC):
                        g = t // 2
                        nc.tensor.matmul(
                            U_ps[0:64,
                                 64 * (t % 2) + 4 * g:64 * (t % 2) + 4 * g + 4],
                            qf_tiles[t // 8][:, 64 * (t % 8):64 * (t % 8) + 64],
                            sf_t[:, 4 * t:4 * t + 4],
                            start=True, stop=True)
                    out_base = out_d[:]
                    for half in range(2):
                        U_sb = wk.tile([64, Q], F32, tag=f"usb{half}",
                                       name=f"usb{half}")
                        nc.vector.tensor_copy(
                            out=U_sb, in_=U_ps[0:64, 64 * half:64 * half + 64])
                        dst = _b.AP(tensor=out_base.tensor,
                                    offset=out_base.offset + 256 * half,
                                    ap=[[4, 64], [512, 16], [1, 4]])
                        nc.sync.dma_start(
                            out=dst, in_=U_sb.rearrange("p (g b) -> p g b", b=4))

    lp.__exit__(None, None, None)
    nc.compile()
    return nc


def _host_prep(inputs):
    f32 = np.float32
    Wi, Wt = np.asarray(inputs["Wi"], f32), np.asarray(inputs["Wt"], f32)
    bi, bt = np.asarray(inputs["bi"], f32), np.asarray(inputs["bt"], f32)
    g1, b1 = np.asarray(inputs["g1"], f32), np.asarray(inputs["b1"], f32)
    g2, b2 = np.asarray(inputs["g2"], f32), np.asarray(inputs["b2"], f32)
    Wq, bq = np.asarray(inputs["Wq"], f32), np.asarray(inputs["bq"], f32)
    Wk, bk = np.asarray(inputs["Wk"], f32), np.asarray(inputs["bk"], f32)
    Wv, bv = np.asarray(inputs["Wv"], f32), np.asarray(inputs["bv"], f32)
    Wo, bo = np.asarray(inputs["Wo"], f32), np.asarray(inputs["bo"], f32)

    Wc = np.concatenate([Wi, Wt], axis=1)          # [128, 2816]
    bc = bi + bt
    Wc_c = Wc - Wc.mean(axis=0, keepdims=True)     # fold LN1 mean
    bc_c = bc - bc.mean()

    Wq_f = (Wq * g1[None, :]) * SCALE_INV
    bq_f = (bq + Wq @ b1) * SCALE_INV
    Wk_f = Wk * g1[None, :]
    bk_f = bk + Wk @ b1
    Wv_f = Wv * g1[None, :]
    bv_f = bv + Wv @ b1
    Wv_c = Wv_f - Wv_f.mean(axis=0, keepdims=True)  # fold LN2 mean
    bv_c = bv_f - bv_f.mean()
    Wo_f = Wo * g2[None, :]
    bo_f = bo + Wo @ b2

    blk = np.arange(SROWS) // S
    mask = np.where(blk[:, None] == blk[None, :], 0.0, -1e30).astype(f32)

    wqkvo = np.concatenate([Wq_f.T, Wk_f.T, Wv_c.T, Wo_f.T], axis=1)
    biases = np.stack([bc_c, bq_f, bk_f, bv_c, bo_f,
                       np.ones(HID, f32)], axis=1)
    common = {
        "wc": np.ascontiguousarray(Wc_c.T),
        "wqkvo": np.ascontiguousarray(wqkvo),
        "biases": np.ascontiguousarray(biases),
        "onesr": np.ones((1, HID), f32),
        "mask": mask, "ident": np.eye(128, dtype=f32),
    }

    si = np.asarray(inputs["support_images"], f32)
    st = np.asarray(inputs["support_texts"], f32)
    qi = np.asarray(inputs["query_images"], f32)
    qt = np.asarray(inputs["query_texts"], f32)

    in_maps = []
    for m in range(NCORES):
        ts = slice(m * TPC, (m + 1) * TPC)
        Xq = np.concatenate([qi[ts].reshape(QROWS, DI),
                             qt[ts].reshape(QROWS, DTXT)], axis=1)
        Xs = np.concatenate([si[ts].reshape(SROWS, DI),
                             st[ts].reshape(SROWS, DTXT)], axis=1)
        X = np.concatenate([Xs, Xq], axis=0)        # [2176, 2816] support first
        xT = np.ascontiguousarray(X.T)              # [2816, 2176]
        in_maps.append({"xT": xT, **common})
    return in_maps


def _run(in_maps, trace=False, **kw):
    from concourse.bass_utils import run_bass_kernel_spmd
    global _prog
    if _prog is None:
        _prog = _build()
    return run_bass_kernel_spmd(_prog, in_maps, list(range(NCORES)),
                                trace=trace, **kw)


def kernel(**inputs) -> np.ndarray:
    in_maps = _host_prep(inputs)
    res = _run(in_maps)
    return np.concatenate([res.results[m]["logits"] for m in range(NCORES)],
                          axis=0)



# revision 6
# speedup vs baseline: 1.5071x; 1.5071x over previous
"""Trainium2 Bass kernel for nn_MetaLearner (meta-learning attention + cosine
prototype scoring), data-parallel over tasks on 8 NeuronCores.

Math (per task):
  c   = [img, txt] @ Wc.T + bc                (Wc = concat(Wi, Wt))
  h   = LN1(c);  q,k,v = h @ W{q,k,v}.T + b   (queries: seqlen=1 -> ctx = v)
  ctx = softmax(q k^T / sqrt(128)) v          (support: seqlen=4)
  f   = LN2(ctx) @ Wo.T + bo
  logits[t,q,c] = 10 * cos(qf[t,q], sf[t,c])

Key tricks (all exact or within f16 rounding):
  - LN gains/biases and mean-subtractions folded into weights on host.
  - Softmax denominator (and max-subtraction) dropped entirely: LN2's
    per-column rstd cancels any positive per-column scale of ctx, and
    mean-centering is already folded into Wv.
  - rsqrt computed as Exp(-0.5*Ln(x)) on ScalarE: one ACT table set for
    the whole kernel (exp also serves softmax), no slow DVE reciprocals.
  - No transposes: attention scores computed pre-transposed (sT = kT^T qT)
    and v computed pre-transposed (vn = h^T WvT) by operand swapping.
  - All PE operands fp16 (1 cyc/row); f32 accumulation in PSUM.
  - Inputs streamed as fp16, host-packed per column-group so every stream
    DMA is a single contiguous 1.2-1.8 MB transfer.
On-chip layout is "transposed" throughout: activations are [hid, rows].
"""
import sys
sys.path.insert(0, "/opt/trn_rl_repo")
import numpy as np

HID = 128
T, Q, S = 256, 64, 4
DI, DTXT = 2048, 768
NCORES = 8
TPC = T // NCORES               # 32 tasks per core
FEAT = DI + DTXT                # 2816
KT = FEAT // 128                # 22 contraction chunks
QROWS = TPC * Q                 # 2048 query rows per core
SROWS = TPC * S                 # 128 support rows per core
ROWS = QROWS + SROWS            # 2176
SCALE_INV = 1.0 / (np.sqrt(HID) + 1e-8)
EPS = 1e-5

_prog = None  # cached compiled Bass program


def _build():
    import concourse.bacc as bacc
    import concourse.tile as tile
    import concourse.mybir as mybir
    import concourse.bass as _b

    F32 = mybir.dt.float32
    F16 = mybir.dt.float16
    AFT = mybir.ActivationFunctionType

    nc = bacc.Bacc()
    x1_d = nc.declare_dram_parameter("x1", [128, KT * 640], F16, isOutput=False)
    x2_d = nc.declare_dram_parameter("x2", [128, KT * 1024], F16, isOutput=False)
    x3_d = nc.declare_dram_parameter("x3", [128, KT * 512], F16, isOutput=False)
    wc_d = nc.declare_dram_parameter("wc", [128, KT * 128], F16, isOutput=False)
    wqkvo_d = nc.declare_dram_parameter("wqkvo", [128, 4 * HID], F16,
                                        isOutput=False)
    bias_d = nc.declare_dram_parameter("biases", [HID, 5], F32, isOutput=False)
    bvrow_d = nc.declare_dram_parameter("bvrow", [1, HID], F16, isOutput=False)
    ones_d = nc.declare_dram_parameter("ones", [1, 640], F16, isOutput=False)
    onesc_d = nc.declare_dram_parameter("onesc", [128, 1], F16, isOutput=False)
    mask_d = nc.declare_dram_parameter("mask01", [SROWS, SROWS], F16,
                                       isOutput=False)
    out_d = nc.declare_dram_parameter("logits", [TPC, Q, S], F32, isOutput=True)

    lp = nc.allow_low_precision(reason="fp16 streaming with f32 accumulation")
    lp.__enter__()

    with tile.TileContext(nc) as tc:
        with (
            tc.tile_pool(name="wts", bufs=1) as wts,
            tc.tile_pool(name="xg1p", bufs=1) as xg1p,
            tc.tile_pool(name="xg2p", bufs=1) as xg2p,
            tc.tile_pool(name="xg3p", bufs=1) as xg3p,
            tc.tile_pool(name="qfp", bufs=1) as qfp,
            tc.tile_pool(name="wk", bufs=2) as wk,
            tc.tile_pool(name="pst", bufs=1, space="PSUM") as pst,
        ):
            # ---- loads, in stream order ----
            wc_t = wts.tile([128, KT, 128], F16)
            nc.sync.dma_start(out=wc_t, in_=wc_d[:])
            wqkvo_t = wts.tile([128, 4 * HID], F16)
            nc.sync.dma_start(out=wqkvo_t, in_=wqkvo_d[:])
            wq_t = wqkvo_t[:, 0 * HID:1 * HID]
            wk_t = wqkvo_t[:, 1 * HID:2 * HID]
            wv_t = wqkvo_t[:, 2 * HID:3 * HID]
            wo_t = wqkvo_t[:, 3 * HID:4 * HID]
            bias_t = wts.tile([HID, 5], F32)
            nc.sync.dma_start(out=bias_t, in_=bias_d[:])
            bc_a, bq_a, bk_a, bv_a, bo_a = (bias_t[:, i:i + 1] for i in range(5))
            bvrow_t = wts.tile([1, HID], F16)
            nc.sync.dma_start(out=bvrow_t, in_=bvrow_d[:])
            ones_t = wts.tile([1, 640], F16)
            nc.sync.dma_start(out=ones_t, in_=ones_d[:])
            ones_r = ones_t[:, 0:128]            # lhsT [K=1, M=128] for bcast
            onesc_t = wts.tile([128, 1], F16)    # lhsT [K=128, M=1] for reduce
            nc.sync.dma_start(out=onesc_t, in_=onesc_d[:])
            mask_t = wts.tile([SROWS, SROWS], F16)
            nc.sync.dma_start(out=mask_t, in_=mask_d[:])
            eps_t = wts.tile([1, 3], F32)
            nc.vector.memset(eps_t[:, 0:1], EPS)
            nc.vector.memset(eps_t[:, 1:2], 1e-16)
            nc.vector.memset(eps_t[:, 2:3], 1e-18)
            eps_ap = {EPS: eps_t[:, 0:1], 1e-16: eps_t[:, 1:2],
                      1e-18: eps_t[:, 2:3]}

            xg1_t = xg1p.tile([128, KT, 640], F16)
            nc.sync.dma_start(out=xg1_t[:, 0:11, :], in_=x1_d[:, 0:11 * 640])
            nc.sync.dma_start(out=xg1_t[:, 11:KT, :], in_=x1_d[:, 11 * 640:])
            xg2_t = xg2p.tile([128, KT, 1024], F16)
            for (a, b) in ((0, 6), (6, 11), (11, 17), (17, KT)):
                nc.sync.dma_start(out=xg2_t[:, a:b, :],
                                  in_=x2_d[:, a * 1024:b * 1024])
            xg3_t = xg3p.tile([128, KT, 512], F16)
            nc.sync.dma_start(out=xg3_t[:, 0:11, :], in_=x3_d[:, 0:11 * 512])
            nc.sync.dma_start(out=xg3_t[:, 11:KT, :], in_=x3_d[:, 11 * 512:])

            qf_t = qfp.tile([128, QROWS], F16)
            sf_t = qfp.tile([128, SROWS], F16)
            U_sb = qfp.tile([64, 2 * Q], F32)

            def rstd(sq_sb, cn, scale, eps):
                """[1/sqrt(scale*colsum(sq)+eps)] broadcast to [128,cn] PSUM."""
                ss_ps = pst.tile([1, 512], F32, tag="ss", bufs=1)
                nc.tensor.matmul(ss_ps[:, :cn], onesc_t[:], sq_sb[:, :cn],
                                 start=True, stop=True)
                ln_r = wk.tile([1, 512], F32, tag="lnr")
                nc.scalar.activation(out=ln_r[:, :cn], in_=ss_ps[:, :cn],
                                     func=AFT.Ln, bias=eps_ap[eps], scale=scale)
                rr = wk.tile([1, 512], F16, tag="rr")
                nc.scalar.activation(out=rr[:, :cn], in_=ln_r[:, :cn],
                                     func=AFT.Exp, bias=0.0, scale=-0.5)
                R_ps = pst.tile([128, 512], F32, tag="R", bufs=2)
                nc.tensor.matmul(R_ps[:, :cn], ones_r, rr[:, :cn],
                                 start=True, stop=True)
                return R_ps

            def query_tail(c_ps, qf_off, cn):
                """c[:, :cn] -> qf_t[:, qf_off:qf_off+cn] + scoring."""
                cf = wk.tile([128, 512], F16, tag="cf")
                nc.vector.tensor_scalar_add(out=cf[:, :cn], in0=c_ps[:, :cn],
                                            scalar1=bc_a)
                sq = wk.tile([128, 512], F16, tag="sq")
                nc.vector.tensor_mul(out=sq[:, :cn], in0=cf[:, :cn],
                                     in1=cf[:, :cn])
                R1 = rstd(sq, cn, 1.0 / HID, EPS)
                h = wk.tile([128, 512], F16, tag="h")
                nc.vector.tensor_mul(out=h[:, :cn], in0=cf[:, :cn],
                                     in1=R1[:, :cn])
                v_ps = pst.tile([128, 512], F32, tag="pp", bufs=2)
                nc.tensor.matmul(v_ps[:, :cn], wv_t, h[:, :cn],
                                 start=True, stop=True)
                vf = wk.tile([128, 512], F16, tag="vf")
                nc.vector.tensor_scalar_add(out=vf[:, :cn], in0=v_ps[:, :cn],
                                            scalar1=bv_a)
                sq2 = wk.tile([128, 512], F16, tag="sq")
                nc.vector.tensor_mul(out=sq2[:, :cn], in0=vf[:, :cn],
                                     in1=vf[:, :cn])
                R2 = rstd(sq2, cn, 1.0 / HID, EPS)
                z = wk.tile([128, 512], F16, tag="h")
                nc.vector.tensor_mul(out=z[:, :cn], in0=vf[:, :cn],
                                     in1=R2[:, :cn])
                o_ps = pst.tile([128, 512], F32, tag="pp", bufs=2)
                nc.tensor.matmul(o_ps[:, :cn], wo_t, z[:, :cn],
                                 start=True, stop=True)
                ff = wk.tile([128, 512], F16, tag="ff")
                nc.vector.tensor_scalar_add(out=ff[:, :cn], in0=o_ps[:, :cn],
                                            scalar1=bo_a)
                sq3 = wk.tile([128, 512], F16, tag="sq")
                nc.vector.tensor_mul(out=sq3[:, :cn], in0=ff[:, :cn],
                                     in1=ff[:, :cn])
                RN = rstd(sq3, cn, 1.0, 1e-16)
                nc.vector.tensor_mul(out=qf_t[:, qf_off:qf_off + cn],
                                     in0=ff[:, :cn], in1=RN[:, :cn])
                # scoring: 8 tasks per 512-wide chunk (needs sf_t)
                g0 = qf_off // 512
                U_ps = pst.tile([64, 32], F32, tag="sc", bufs=1)
                for j in range(8):
                    t = 8 * g0 + j
                    nc.tensor.matmul(U_ps[0:64, 4 * j:4 * j + 4],
                                     qf_t[:, 64 * t:64 * t + 64],
                                     sf_t[:, 4 * t:4 * t + 4],
                                     start=True, stop=True)
                nc.vector.tensor_copy(out=U_sb[0:64, 32 * g0:32 * g0 + 32],
                                      in_=U_ps[0:64, 0:32])

            def support_tail(cS_ps):
                cfS = wk.tile([128, SROWS], F16, tag="cfS")
                nc.vector.tensor_scalar_add(out=cfS, in0=cS_ps[:, 0:SROWS],
                                            scalar1=bc_a)
                sqS = wk.tile([128, SROWS], F16, tag="sqS")
                nc.vector.tensor_mul(out=sqS, in0=cfS, in1=cfS)
                R1 = rstd(sqS, SROWS, 1.0 / HID, EPS)
                hs = wk.tile([128, SROWS], F16, tag="hs")
                nc.vector.tensor_mul(out=hs, in0=cfS, in1=R1[:, :SROWS])
                q_ps = pst.tile([128, 512], F32, tag="pp", bufs=2)
                nc.tensor.matmul(q_ps[:, :SROWS], wq_t, hs, start=True, stop=True)
                qTb = wk.tile([128, SROWS], F16, tag="qTb")
                nc.vector.tensor_scalar_add(out=qTb, in0=q_ps[:, :SROWS],
                                            scalar1=bq_a)
                k_ps = pst.tile([128, 512], F32, tag="pp", bufs=2)
                nc.tensor.matmul(k_ps[:, :SROWS], wk_t, hs, start=True, stop=True)
                kTb = wk.tile([128, SROWS], F16, tag="kTb")
                nc.vector.tensor_scalar_add(out=kTb, in0=k_ps[:, :SROWS],
                                            scalar1=bk_a)
                # vn = (Wv h)^T + 1 (x) bv  : [srows, hid]
                vn_ps = pst.tile([128, 512], F32, tag="pp", bufs=2)
                nc.tensor.matmul(vn_ps[:, :HID], hs, wv_t, start=True, stop=False)
                nc.tensor.matmul(vn_ps[:, :HID], ones_t[:, 0:SROWS], bvrow_t[:],
                                 start=False, stop=True)
                vn_sb = wk.tile([SROWS, HID], F16, tag="vn")
                nc.vector.tensor_copy(out=vn_sb, in_=vn_ps[:, :HID])
                # sT[k_row, q_row] = k^T q ; unnormalized masked exp
                sT_ps = pst.tile([128, 512], F32, tag="pp", bufs=2)
                nc.tensor.matmul(sT_ps[:, :SROWS], kTb, qTb, start=True, stop=True)
                aT = wk.tile([SROWS, SROWS], F16, tag="aT")
                nc.scalar.activation(out=aT, in_=sT_ps[:, :SROWS], func=AFT.Exp,
                                     bias=0.0, scale=1.0)
                am = wk.tile([SROWS, SROWS], F16, tag="am")
                nc.vector.tensor_mul(out=am, in0=aT, in1=mask_t)
                ctx_ps = pst.tile([128, 512], F32, tag="pp", bufs=2)
                nc.tensor.matmul(ctx_ps[:, :SROWS], vn_sb, am, start=True,
                                 stop=True)
                cfx = wk.tile([128, SROWS], F16, tag="cfS")
                nc.vector.tensor_copy(out=cfx, in_=ctx_ps[:, :SROWS])
                sqx = wk.tile([128, SROWS], F16, tag="sqS")
                nc.vector.tensor_mul(out=sqx, in0=cfx, in1=cfx)
                R2 = rstd(sqx, SROWS, 1.0 / HID, EPS)
                zx = wk.tile([128, SROWS], F16, tag="hs")
                nc.vector.tensor_mul(out=zx, in0=cfx, in1=R2[:, :SROWS])
                o_ps = pst.tile([128, 512], F32, tag="pp", bufs=2)
                nc.tensor.matmul(o_ps[:, :SROWS], wo_t, zx, start=True, stop=True)
                ffx = wk.tile([128, SROWS], F16, tag="ffS")
                nc.vector.tensor_scalar_add(out=ffx, in0=o_ps[:, :SROWS],
                                            scalar1=bo_a)
                sqf = wk.tile([128, SROWS], F16, tag="sqS")
                nc.vector.tensor_mul(out=sqf, in0=ffx, in1=ffx)
                RN = rstd(sqf, SROWS, 0.01, 1e-18)   # = 10/||f||, clipped
                nc.vector.tensor_mul(out=sf_t[:], in0=ffx, in1=RN[:, :SROWS])

            # ---- group 1: support + q0 ----
            with tc.tile_pool(name="psA", bufs=1, space="PSUM") as psA:
                cS_ps = psA.tile([128, SROWS], F32)
                c0_ps = psA.tile([128, 512], F32)
                for k in range(KT):
                    nc.tensor.matmul(cS_ps[:], wc_t[:, k, :],
                                     xg1_t[:, k, 0:128],
                                     start=(k == 0), stop=(k == KT - 1))
                    nc.tensor.matmul(c0_ps[:], wc_t[:, k, :],
                                     xg1_t[:, k, 128:640],
                                     start=(k == 0), stop=(k == KT - 1))
                support_tail(cS_ps)
                query_tail(c0_ps, 0, 512)

            # ---- group 2: q1 + q2 ----
            with tc.tile_pool(name="psB", bufs=1, space="PSUM") as psB:
                c1_ps = psB.tile([128, 512], F32)
                c2_ps = psB.tile([128, 512], F32)
                for k in range(KT):
                    nc.tensor.matmul(c1_ps[:], wc_t[:, k, :],
                                     xg2_t[:, k, 0:512],
                                     start=(k == 0), stop=(k == KT - 1))
                    nc.tensor.matmul(c2_ps[:], wc_t[:, k, :],
                                     xg2_t[:, k, 512:1024],
                                     start=(k == 0), stop=(k == KT - 1))
                query_tail(c1_ps, 512, 512)
                query_tail(c2_ps, 1024, 512)

            # ---- group 3: q3 ----
            with tc.tile_pool(name="psC", bufs=1, space="PSUM") as psC:
                c3_ps = psC.tile([128, 512], F32)
                for k in range(KT):
                    nc.tensor.matmul(c3_ps[:], wc_t[:, k, :], xg3_t[:, k, :],
                                     start=(k == 0), stop=(k == KT - 1))
                query_tail(c3_ps, 1536, 512)

            # ---- output: logits[t, q, s] = U_sb[q, 4t+s] ----
            out_base = out_d[:]
            dst = _b.AP(tensor=out_base.tensor, offset=out_base.offset,
                        ap=[[4, 64], [256, 32], [1, 4]])
            nc.sync.dma_start(out=dst,
                              in_=U_sb.rearrange("p (g b) -> p g b", b=4))

    lp.__exit__(None, None, None)
    nc.compile()
    return nc


def _host_prep(inputs):
    f32, f16 = np.float32, np.float16
    Wi, Wt = np.asarray(inputs["Wi"], f32), np.asarray(inputs["Wt"], f32)
    bi, bt = np.asarray(inputs["bi"], f32), np.asarray(inputs["bt"], f32)
    g1, b1 = np.asarray(inputs["g1"], f32), np.asarray(inputs["b1"], f32)
    g2, b2 = np.asarray(inputs["g2"], f32), np.asarray(inputs["b2"], f32)
    Wq, bq = np.asarray(inputs["Wq"], f32), np.asarray(inputs["bq"], f32)
    Wk, bk = np.asarray(inputs["Wk"], f32), np.asarray(inputs["bk"], f32)
    Wv, bv = np.asarray(inputs["Wv"], f32), np.asarray(inputs["bv"], f32)
    Wo, bo = np.asarray(inputs["Wo"], f32), np.asarray(inputs["bo"], f32)

    Wc = np.concatenate([Wi, Wt], axis=1)          # [128, 2816]
    bc = bi + bt
    Wc_c = Wc - Wc.mean(axis=0, keepdims=True)     # fold LN1 mean
    bc_c = bc - bc.mean()

    Wq_f = (Wq * g1[None, :]) * SCALE_INV
    bq_f = (bq + Wq @ b1) * SCALE_INV
    Wk_f = Wk * g1[None, :]
    bk_f = bk + Wk @ b1
    Wv_f = Wv * g1[None, :]
    bv_f = bv + Wv @ b1
    Wv_c = Wv_f - Wv_f.mean(axis=0, keepdims=True)  # fold LN2 mean
    bv_c = bv_f - bv_f.mean()
    Wo_f = Wo * g2[None, :]
    bo_f = bo + Wo @ b2

    blk = np.arange(SROWS) // S
    mask01 = (blk[:, None] == blk[None, :]).astype(f16)

    def pack_kmajor(a):   # [feat, cols] -> [128, KT*cols] (p, k, c)
        cols = a.shape[1]
        return np.ascontiguousarray(
            a.reshape(KT, 128, cols).transpose(1, 0, 2).reshape(128, -1)
        )

    common = {
        "wc": pack_kmajor(Wc_c.T.astype(f16)),
        "wqkvo": np.concatenate([Wq_f.T, Wk_f.T, Wv_c.T, Wo_f.T],
                                axis=1).astype(f16),
        "biases": np.ascontiguousarray(
            np.stack([bc_c, bq_f, bk_f, bv_c, bo_f], axis=1)),
        "bvrow": np.ascontiguousarray(bv_c[None, :]).astype(f16),
        "ones": np.ones((1, 640), f16),
        "onesc": np.ones((128, 1), f16),
        "mask01": mask01,
    }

    si = np.asarray(inputs["support_images"], f32)
    st = np.asarray(inputs["support_texts"], f32)
    qi = np.asarray(inputs["query_images"], f32)
    qt = np.asarray(inputs["query_texts"], f32)

    in_maps = []
    for m in range(NCORES):
        ts = slice(m * TPC, (m + 1) * TPC)
        Xq = np.concatenate([qi[ts].reshape(QROWS, DI),
                             qt[ts].reshape(QROWS, DTXT)], axis=1)
        Xs = np.concatenate([si[ts].reshape(SROWS, DI),
                             st[ts].reshape(SROWS, DTXT)], axis=1)
        xT = np.concatenate([Xs, Xq], axis=0).T.astype(f16)  # [2816, 2176]
        in_maps.append({
            "x1": pack_kmajor(xT[:, 0:640]),
            "x2": pack_kmajor(xT[:, 640:1664]),
            "x3": pack_kmajor(xT[:, 1664:2176]),
            **common,
        })
    return in_maps


def _run(in_maps, trace=False, **kw):
    from concourse.bass_utils import run_bass_kernel_spmd
    global _prog
    if _prog is None:
        _prog = _build()
    return run_bass_kernel_spmd(_prog, in_maps, list(range(NCORES)),
                                trace=trace, **kw)


def kernel(**inputs) -> np.ndarray:
    in_maps = _host_prep(inputs)
    res = _run(in_maps)
    return np.concatenate([res.results[m]["logits"] for m in range(NCORES)],
                          axis=0)


# revision 7
# speedup vs baseline: 2.7318x; 1.8126x over previous
"""Trainium2 Bass kernel for nn_MetaLearner (meta-learning attention + cosine
prototype scoring), data-parallel over tasks on 8 NeuronCores.

Math (per task):
  c   = [img, txt] @ Wc.T + bc                (Wc = concat(Wi, Wt))
  h   = LN1(c);  q,k,v = h @ W{q,k,v}.T + b   (queries: seqlen=1 -> ctx = v)
  ctx = softmax(q k^T / sqrt(128)) v          (support: seqlen=4)
  f   = LN2(ctx) @ Wo.T + bo
  logits[t,q,c] = 10 * cos(qf[t,q], sf[t,c])

Key tricks:
  - LN gains/biases and mean-subtractions folded into weights on host.
  - Softmax denominator (and max-subtraction) dropped: LN2's rstd cancels
    any positive per-column scale of ctx; mean-centering folded into Wv.
  - When bc/bv/bo fold to zero (true for the reference initialization),
    every per-column scale cancels through the final cosine normalize, so
    the query path collapses to qf = normalize((Wo Wv_c) c) and support
    skips its LN2 rstd. A general fallback path keeps the full math.
  - rstd = Sqrt(reciprocal_approx_fast(scale * colsum(x^2))): the scale
    rides in the reduction weights, DVE does the reciprocal, ACT only ever
    evaluates Sqrt (plus one Exp for softmax) -> ~2 table switches total.
  - No transposes: attention scores computed pre-transposed (sT = kT^T qT)
    and v computed pre-transposed (vn = h^T WvT) by operand swapping.
  - All PE operands fp16 (1 cyc/row); f32 accumulation in PSUM.
  - Inputs streamed as fp16, host-packed per column-group so every stream
    DMA is a single contiguous 1.2-1.8 MB transfer.
On-chip layout is "transposed" throughout: activations are [hid, rows].
"""
import sys
sys.path.insert(0, "/opt/trn_rl_repo")
import numpy as np

HID = 128
T, Q, S = 256, 64, 4
DI, DTXT = 2048, 768
NCORES = 8
TPC = T // NCORES               # 32 tasks per core
FEAT = DI + DTXT                # 2816
KT = FEAT // 128                # 22 contraction chunks
QROWS = TPC * Q                 # 2048 query rows per core
SROWS = TPC * S                 # 128 support rows per core
ROWS = QROWS + SROWS            # 2176
SCALE_INV = 1.0 / (np.sqrt(HID) + 1e-8)

_progs = {}  # cached compiled Bass programs, keyed by fast-path flag


def _build(fast):
    import concourse.bacc as bacc
    import concourse.tile as tile
    import concourse.mybir as mybir
    import concourse.bass as _b

    F32 = mybir.dt.float32
    F16 = mybir.dt.float16
    AFT = mybir.ActivationFunctionType

    nc = bacc.Bacc()
    x1_d = nc.declare_dram_parameter("x1", [128, KT * 640], F16, isOutput=False)
    x2_d = nc.declare_dram_parameter("x2", [128, KT * 1024], F16, isOutput=False)
    x3_d = nc.declare_dram_parameter("x3", [128, KT * 512], F16, isOutput=False)
    wc_d = nc.declare_dram_parameter("wc", [128, KT * 128], F16, isOutput=False)
    wqkvo_d = nc.declare_dram_parameter("wqkvo", [128, 5 * HID], F16,
                                        isOutput=False)
    bias_d = nc.declare_dram_parameter("biases", [HID, 5], F32, isOutput=False)
    bvrow_d = nc.declare_dram_parameter("bvrow", [1, HID], F16, isOutput=False)
    ones_d = nc.declare_dram_parameter("ones", [1, 640], F16, isOutput=False)
    onesc_d = nc.declare_dram_parameter("onesc", [128, 4], F16, isOutput=False)
    mask_d = nc.declare_dram_parameter("mask01", [SROWS, SROWS], F16,
                                       isOutput=False)
    out_d = nc.declare_dram_parameter("logits", [TPC, Q, S], F32, isOutput=True)

    lp = nc.allow_low_precision(reason="fp16 streaming with f32 accumulation")
    lp.__enter__()

    with tile.TileContext(nc) as tc:
        with (
            tc.tile_pool(name="wts", bufs=1) as wts,
            tc.tile_pool(name="xg1p", bufs=1) as xg1p,
            tc.tile_pool(name="xg2p", bufs=1) as xg2p,
            tc.tile_pool(name="xg3p", bufs=1) as xg3p,
            tc.tile_pool(name="qfp", bufs=1) as qfp,
            tc.tile_pool(name="wk", bufs=2) as wk,
            tc.tile_pool(name="pst", bufs=1, space="PSUM") as pst,
        ):
            # ---- loads, in stream order ----
            wc_t = wts.tile([128, KT, 128], F16)
            nc.sync.dma_start(out=wc_t, in_=wc_d[:])
            wqkvo_t = wts.tile([128, 5 * HID], F16)
            nc.sync.dma_start(out=wqkvo_t, in_=wqkvo_d[:])
            wq_t = wqkvo_t[:, 0 * HID:1 * HID]
            wk_t = wqkvo_t[:, 1 * HID:2 * HID]
            wv_t = wqkvo_t[:, 2 * HID:3 * HID]
            wo_t = wqkvo_t[:, 3 * HID:4 * HID]
            wov_t = wqkvo_t[:, 4 * HID:5 * HID]
            bias_t = wts.tile([HID, 5], F32)
            nc.sync.dma_start(out=bias_t, in_=bias_d[:])
            bc_a, bq_a, bk_a, bv_a, bo_a = (bias_t[:, i:i + 1] for i in range(5))
            bvrow_t = wts.tile([1, HID], F16)
            nc.sync.dma_start(out=bvrow_t, in_=bvrow_d[:])
            ones_t = wts.tile([1, 640], F16)
            nc.sync.dma_start(out=ones_t, in_=ones_d[:])
            ones_r = ones_t[:, 0:128]            # lhsT [K=1, M=128] for bcast
            onesc_t = wts.tile([128, 4], F16)    # reduce lhsT columns: scales
            nc.sync.dma_start(out=onesc_t, in_=onesc_d[:])
            red_ln = onesc_t[:, 0:1]             # 1/128
            red_q = onesc_t[:, 1:2]              # 1.0
            red_s = onesc_t[:, 2:3]              # 0.01
            mask_t = wts.tile([SROWS, SROWS], F16)
            nc.sync.dma_start(out=mask_t, in_=mask_d[:])

            xg1_t = xg1p.tile([128, KT, 640], F16)
            nc.sync.dma_start(out=xg1_t[:, 0:11, :], in_=x1_d[:, 0:11 * 640])
            nc.sync.dma_start(out=xg1_t[:, 11:KT, :], in_=x1_d[:, 11 * 640:])
            xg2_t = xg2p.tile([128, KT, 1024], F16)
            for (a, b) in ((0, 6), (6, 11), (11, 17), (17, KT)):
                nc.sync.dma_start(out=xg2_t[:, a:b, :],
                                  in_=x2_d[:, a * 1024:b * 1024])
            xg3_t = xg3p.tile([128, KT, 512], F16)
            nc.sync.dma_start(out=xg3_t[:, 0:11, :], in_=x3_d[:, 0:11 * 512])
            nc.sync.dma_start(out=xg3_t[:, 11:KT, :], in_=x3_d[:, 11 * 512:])

            qf_t = qfp.tile([128, QROWS], F16)
            sf_t = qfp.tile([128, SROWS], F16)
            U_sb = qfp.tile([64, 2 * Q], F32)

            def rstd(sq_sb, cn, red):
                """1/sqrt(red . sq) broadcast to [128, cn] PSUM (f16 path)."""
                ss_ps = pst.tile([1, 512], F32, tag="ss", bufs=1)
                nc.tensor.matmul(ss_ps[:, :cn], red, sq_sb[:, :cn],
                                 start=True, stop=True)
                ir = wk.tile([1, 512], F32, tag="ir")
                nc.vector.reciprocal_approx_fast(out=ir[:, :cn],
                                                 in_=ss_ps[:, :cn])
                rr = wk.tile([1, 512], F16, tag="rr")
                nc.scalar.activation(out=rr[:, :cn], in_=ir[:, :cn],
                                     func=AFT.Sqrt, bias=0.0, scale=1.0)
                R_ps = pst.tile([128, 512], F32, tag="R", bufs=2)
                nc.tensor.matmul(R_ps[:, :cn], ones_r, rr[:, :cn],
                                 start=True, stop=True)
                return R_ps

            def score(g0):
                U_ps = pst.tile([64, 32], F32, tag="sc", bufs=1)
                for j in range(8):
                    t = 8 * g0 + j
                    nc.tensor.matmul(U_ps[0:64, 4 * j:4 * j + 4],
                                     qf_t[:, 64 * t:64 * t + 64],
                                     sf_t[:, 4 * t:4 * t + 4],
                                     start=True, stop=True)
                nc.vector.tensor_copy(out=U_sb[0:64, 32 * g0:32 * g0 + 32],
                                      in_=U_ps[0:64, 0:32])

            def query_tail_fast(c_ps, qf_off, cn):
                """qf = normalize(Wov c); valid when bc=bv=bo=0."""
                cf = wk.tile([128, 512], F16, tag="cf")
                nc.vector.tensor_copy(out=cf[:, :cn], in_=c_ps[:, :cn])
                u_ps = pst.tile([128, 512], F32, tag="pp", bufs=2)
                nc.tensor.matmul(u_ps[:, :cn], wov_t, cf[:, :cn],
                                 start=True, stop=True)
                ff = wk.tile([128, 512], F16, tag="ff")
                nc.vector.tensor_copy(out=ff[:, :cn], in_=u_ps[:, :cn])
                sq = wk.tile([128, 512], F16, tag="sq")
                nc.vector.tensor_mul(out=sq[:, :cn], in0=ff[:, :cn],
                                     in1=ff[:, :cn])
                RN = rstd(sq, cn, red_q)
                nc.vector.tensor_mul(out=qf_t[:, qf_off:qf_off + cn],
                                     in0=ff[:, :cn], in1=RN[:, :cn])
                score(qf_off // 512)

            def query_tail_gen(c_ps, qf_off, cn):
                cf = wk.tile([128, 512], F16, tag="cf")
                nc.vector.tensor_scalar_add(out=cf[:, :cn], in0=c_ps[:, :cn],
                                            scalar1=bc_a)
                sq = wk.tile([128, 512], F16, tag="sq")
                nc.vector.tensor_mul(out=sq[:, :cn], in0=cf[:, :cn],
                                     in1=cf[:, :cn])
                R1 = rstd(sq, cn, red_ln)
                h = wk.tile([128, 512], F16, tag="h")
                nc.vector.tensor_mul(out=h[:, :cn], in0=cf[:, :cn],
                                     in1=R1[:, :cn])
                v_ps = pst.tile([128, 512], F32, tag="pp", bufs=2)
                nc.tensor.matmul(v_ps[:, :cn], wv_t, h[:, :cn],
                                 start=True, stop=True)
                vf = wk.tile([128, 512], F16, tag="vf")
                nc.vector.tensor_scalar_add(out=vf[:, :cn], in0=v_ps[:, :cn],
                                            scalar1=bv_a)
                sq2 = wk.tile([128, 512], F16, tag="sq")
                nc.vector.tensor_mul(out=sq2[:, :cn], in0=vf[:, :cn],
                                     in1=vf[:, :cn])
                R2 = rstd(sq2, cn, red_ln)
                z = wk.tile([128, 512], F16, tag="h")
                nc.vector.tensor_mul(out=z[:, :cn], in0=vf[:, :cn],
                                     in1=R2[:, :cn])
                o_ps = pst.tile([128, 512], F32, tag="pp", bufs=2)
                nc.tensor.matmul(o_ps[:, :cn], wo_t, z[:, :cn],
                                 start=True, stop=True)
                ff = wk.tile([128, 512], F16, tag="ff")
                nc.vector.tensor_scalar_add(out=ff[:, :cn], in0=o_ps[:, :cn],
                                            scalar1=bo_a)
                sq3 = wk.tile([128, 512], F16, tag="sq")
                nc.vector.tensor_mul(out=sq3[:, :cn], in0=ff[:, :cn],
                                     in1=ff[:, :cn])
                RN = rstd(sq3, cn, red_q)
                nc.vector.tensor_mul(out=qf_t[:, qf_off:qf_off + cn],
                                     in0=ff[:, :cn], in1=RN[:, :cn])
                score(qf_off // 512)

            query_tail = query_tail_fast if fast else query_tail_gen

            def support_tail(cS_ps):
                cfS = wk.tile([128, SROWS], F16, tag="cfS")
                if fast:
                    nc.vector.tensor_copy(out=cfS, in_=cS_ps[:, 0:SROWS])
                else:
                    nc.vector.tensor_scalar_add(out=cfS, in0=cS_ps[:, 0:SROWS],
                                                scalar1=bc_a)
                sqS = wk.tile([128, SROWS], F16, tag="sqS")
                nc.vector.tensor_mul(out=sqS, in0=cfS, in1=cfS)
                R1 = rstd(sqS, SROWS, red_ln)
                hs = wk.tile([128, SROWS], F16, tag="hs")
                nc.vector.tensor_mul(out=hs, in0=cfS, in1=R1[:, :SROWS])
                q_ps = pst.tile([128, 512], F32, tag="pp", bufs=2)
                nc.tensor.matmul(q_ps[:, :SROWS], wq_t, hs, start=True, stop=True)
                qTb = wk.tile([128, SROWS], F16, tag="qTb")
                nc.vector.tensor_scalar_add(out=qTb, in0=q_ps[:, :SROWS],
                                            scalar1=bq_a)
                k_ps = pst.tile([128, 512], F32, tag="pp", bufs=2)
                nc.tensor.matmul(k_ps[:, :SROWS], wk_t, hs, start=True, stop=True)
                kTb = wk.tile([128, SROWS], F16, tag="kTb")
                nc.vector.tensor_scalar_add(out=kTb, in0=k_ps[:, :SROWS],
                                            scalar1=bk_a)
                # vn = (Wv h)^T (+ 1 (x) bv in the general path) : [srows, hid]
                vn_ps = pst.tile([128, 512], F32, tag="pp", bufs=2)
                nc.tensor.matmul(vn_ps[:, :HID], hs, wv_t, start=True,
                                 stop=fast)
                if not fast:
                    nc.tensor.matmul(vn_ps[:, :HID], ones_t[:, 0:SROWS],
                                     bvrow_t[:], start=False, stop=True)
                vn_sb = wk.tile([SROWS, HID], F16, tag="vn")
                nc.vector.tensor_copy(out=vn_sb, in_=vn_ps[:, :HID])
                # sT[k_row, q_row] = k^T q ; unnormalized masked exp
                sT_ps = pst.tile([128, 512], F32, tag="pp", bufs=2)
                nc.tensor.matmul(sT_ps[:, :SROWS], kTb, qTb, start=True,
                                 stop=True)
                aT = wk.tile([SROWS, SROWS], F16, tag="aT")
                nc.scalar.activation(out=aT, in_=sT_ps[:, :SROWS], func=AFT.Exp,
                                     bias=0.0, scale=1.0)
                am = wk.tile([SROWS, SROWS], F16, tag="am")
                nc.vector.tensor_mul(out=am, in0=aT, in1=mask_t)
                ctx_ps = pst.tile([128, 512], F32, tag="pp", bufs=2)
                nc.tensor.matmul(ctx_ps[:, :SROWS], vn_sb, am, start=True,
                                 stop=True)
                cfx = wk.tile([128, SROWS], F16, tag="cfS")
                nc.vector.tensor_copy(out=cfx, in_=ctx_ps[:, :SROWS])
                if fast:
                    zx = cfx           # LN2 rstd cancels in the final normalize
                else:
                    sqx = wk.tile([128, SROWS], F16, tag="sqS")
                    nc.vector.tensor_mul(out=sqx, in0=cfx, in1=cfx)
                    R2 = rstd(sqx, SROWS, red_ln)
                    zx = wk.tile([128, SROWS], F16, tag="hs")
                    nc.vector.tensor_mul(out=zx, in0=cfx, in1=R2[:, :SROWS])
                o_ps = pst.tile([128, 512], F32, tag="pp", bufs=2)
                nc.tensor.matmul(o_ps[:, :SROWS], wo_t, zx, start=True, stop=True)
                ffx = wk.tile([128, SROWS], F16, tag="ffS")
                if fast:
                    nc.vector.tensor_copy(out=ffx, in_=o_ps[:, :SROWS])
                else:
                    nc.vector.tensor_scalar_add(out=ffx, in0=o_ps[:, :SROWS],
                                                scalar1=bo_a)
                sqf = wk.tile([128, SROWS], F16, tag="sqS")
                nc.vector.tensor_mul(out=sqf, in0=ffx, in1=ffx)
                RN = rstd(sqf, SROWS, red_s)   # = 10/||f||
                nc.vector.tensor_mul(out=sf_t[:], in0=ffx, in1=RN[:, :SROWS])

            # ---- group 1: support + q0 ----
            with tc.tile_pool(name="psA", bufs=1, space="PSUM") as psA:
                cS_ps = psA.tile([128, SROWS], F32)
                c0_ps = psA.tile([128, 512], F32)
                for k in range(KT):
                    nc.tensor.matmul(cS_ps[:], wc_t[:, k, :],
                                     xg1_t[:, k, 0:128],
                                     start=(k == 0), stop=(k == KT - 1))
                    nc.tensor.matmul(c0_ps[:], wc_t[:, k, :],
                                     xg1_t[:, k, 128:640],
                                     start=(k == 0), stop=(k == KT - 1))
                support_tail(cS_ps)
                query_tail(c0_ps, 0, 512)

            # ---- group 2: q1 + q2 ----
            with tc.tile_pool(name="psB", bufs=1, space="PSUM") as psB:
                c1_ps = psB.tile([128, 512], F32)
                c2_ps = psB.tile([128, 512], F32)
                for k in range(KT):
                    nc.tensor.matmul(c1_ps[:], wc_t[:, k, :],
                                     xg2_t[:, k, 0:512],
                                     start=(k == 0), stop=(k == KT - 1))
                    nc.tensor.matmul(c2_ps[:], wc_t[:, k, :],
                                     xg2_t[:, k, 512:1024],
                                     start=(k == 0), stop=(k == KT - 1))
                query_tail(c1_ps, 512, 512)
                query_tail(c2_ps, 1024, 512)

            # ---- group 3: q3 ----
            with tc.tile_pool(name="psC", bufs=1, space="PSUM") as psC:
                c3_ps = psC.tile([128, 512], F32)
                for k in range(KT):
                    nc.tensor.matmul(c3_ps[:], wc_t[:, k, :], xg3_t[:, k, :],
                                     start=(k == 0), stop=(k == KT - 1))
                query_tail(c3_ps, 1536, 512)

            # ---- output: logits[t, q, s] = U_sb[q, 4t+s] ----
            out_base = out_d[:]
            dst = _b.AP(tensor=out_base.tensor, offset=out_base.offset,
                        ap=[[4, 64], [256, 32], [1, 4]])
            nc.sync.dma_start(out=dst,
                              in_=U_sb.rearrange("p (g b) -> p g b", b=4))

    lp.__exit__(None, None, None)
    nc.compile()
    return nc


def _host_prep(inputs):
    f32, f16 = np.float32, np.float16
    Wi, Wt = np.asarray(inputs["Wi"], f32), np.asarray(inputs["Wt"], f32)
    bi, bt = np.asarray(inputs["bi"], f32), np.asarray(inputs["bt"], f32)
    g1, b1 = np.asarray(inputs["g1"], f32), np.asarray(inputs["b1"], f32)
    g2, b2 = np.asarray(inputs["g2"], f32), np.asarray(inputs["b2"], f32)
    Wq, bq = np.asarray(inputs["Wq"], f32), np.asarray(inputs["bq"], f32)
    Wk, bk = np.asarray(inputs["Wk"], f32), np.asarray(inputs["bk"], f32)
    Wv, bv = np.asarray(inputs["Wv"], f32), np.asarray(inputs["bv"], f32)
    Wo, bo = np.asarray(inputs["Wo"], f32), np.asarray(inputs["bo"], f32)

    Wc = np.concatenate([Wi, Wt], axis=1)          # [128, 2816]
    bc = bi + bt
    Wc_c = Wc - Wc.mean(axis=0, keepdims=True)     # fold LN1 mean
    bc_c = bc - bc.mean()

    Wq_f = (Wq * g1[None, :]) * SCALE_INV
    bq_f = (bq + Wq @ b1) * SCALE_INV
    Wk_f = Wk * g1[None, :]
    bk_f = bk + Wk @ b1
    Wv_f = Wv * g1[None, :]
    bv_f = bv + Wv @ b1
    Wv_c = Wv_f - Wv_f.mean(axis=0, keepdims=True)  # fold LN2 mean
    bv_c = bv_f - bv_f.mean()
    Wo_f = Wo * g2[None, :]
    bo_f = bo + Wo @ b2
    Wov = Wo_f @ Wv_c                               # fast-path combined proj

    fast = (np.abs(bc_c).max() < 1e-12 and np.abs(bv_c).max() < 1e-12
            and np.abs(bo_f).max() < 1e-12)

    blk = np.arange(SROWS) // S
    mask01 = (blk[:, None] == blk[None, :]).astype(f16)
    onesc = np.zeros((128, 4), f32)
    onesc[:, 0] = 1.0 / HID
    onesc[:, 1] = 1.0
    onesc[:, 2] = 0.01
    onesc[:, 3] = 1.0

    def pack_kmajor(a):   # [feat, cols] -> [128, KT*cols] (p, k, c)
        cols = a.shape[1]
        return np.ascontiguousarray(
            a.reshape(KT, 128, cols).transpose(1, 0, 2).reshape(128, -1)
        )

    common = {
        "wc": pack_kmajor(Wc_c.T.astype(f16)),
        "wqkvo": np.concatenate([Wq_f.T, Wk_f.T, Wv_c.T, Wo_f.T, Wov.T],
                                axis=1).astype(f16),
        "biases": np.ascontiguousarray(
            np.stack([bc_c, bq_f, bk_f, bv_c, bo_f], axis=1)),
        "bvrow": np.ascontiguousarray(bv_c[None, :]).astype(f16),
        "ones": np.ones((1, 640), f16),
        "onesc": onesc.astype(f16),
        "mask01": mask01,
    }

    si = np.asarray(inputs["support_images"], f32)
    st = np.asarray(inputs["support_texts"], f32)
    qi = np.asarray(inputs["query_images"], f32)
    qt = np.asarray(inputs["query_texts"], f32)

    in_maps = []
    for m in range(NCORES):
        ts = slice(m * TPC, (m + 1) * TPC)
        Xq = np.concatenate([qi[ts].reshape(QROWS, DI),
                             qt[ts].reshape(QROWS, DTXT)], axis=1)
        Xs = np.concatenate([si[ts].reshape(SROWS, DI),
                             st[ts].reshape(SROWS, DTXT)], axis=1)
        xT = np.concatenate([Xs, Xq], axis=0).T.astype(f16)  # [2816, 2176]
        in_maps.append({
            "x1": pack_kmajor(xT[:, 0:640]),
            "x2": pack_kmajor(xT[:, 640:1664]),
            "x3": pack_kmajor(xT[:, 1664:2176]),
            **common,
        })
    return in_maps, fast


def _run(in_maps, fast, trace=False, **kw):
    from concourse.bass_utils import run_bass_kernel_spmd
    if fast not in _progs:
        _progs[fast] = _build(fast)
    return run_bass_kernel_spmd(_progs[fast], in_maps, list(range(NCORES)),
                                trace=trace, **kw)


def kernel(**inputs) -> np.ndarray:
    in_maps, fast = _host_prep(inputs)
    res = _run(in_maps, fast)
    return np.concatenate([res.results[m]["logits"] for m in range(NCORES)],
                          axis=0)


# revision 8
# speedup vs baseline: 2.7944x; 1.0229x over previous
"""Trainium2 Bass kernel for nn_MetaLearner (meta-learning attention + cosine
prototype scoring), data-parallel over tasks on 8 NeuronCores.

Math (per task):
  c   = [img, txt] @ Wc.T + bc                (Wc = concat(Wi, Wt))
  h   = LN1(c);  q,k,v = h @ W{q,k,v}.T + b   (queries: seqlen=1 -> ctx = v)
  ctx = softmax(q k^T / sqrt(128)) v          (support: seqlen=4)
  f   = LN2(ctx) @ Wo.T + bo
  logits[t,q,c] = 10 * cos(qf[t,q], sf[t,c])

Key tricks:
  - LN gains/biases and mean-subtractions folded into weights on host.
  - Softmax denominator (and max-subtraction) dropped: LN2's rstd cancels
    any positive per-column scale of ctx; mean-centering folded into Wv.
  - When bc/bv/bo fold to zero (true for the reference initialization),
    every per-column scale cancels through the final cosine normalize, so
    the whole query path collapses to qf = normalize(Wu x) with
    Wu = (Wo Wv_c) Wc_c folded on host -- queries never materialize c, h,
    v, or f; the projection accumulates directly during streaming.
    Support keeps LN1 (softmax is scale-sensitive) but skips LN2's rstd.
    A general fallback path keeps the full math.
  - rstd = Sqrt(reciprocal_approx_fast(scale * colsum(x^2))): the scale
    rides in the reduction weights, DVE does the reciprocal, ACT only ever
    evaluates Sqrt (plus one Exp for softmax) -> ~2 table switches total.
  - No transposes: attention scores computed pre-transposed (sT = kT^T qT)
    and v computed pre-transposed (vn = h^T WvT) by operand swapping.
  - All PE operands fp16 (1 cyc/row); f32 accumulation in PSUM.
  - Inputs streamed as fp16, host-packed per column-group so every stream
    DMA is one contiguous 1.4-5.6 MB transfer. The last group is only 256
    columns so the final (unhidable) tail chain is short.
On-chip layout is "transposed" throughout: activations are [hid, rows].
"""
import sys
sys.path.insert(0, "/opt/trn_rl_repo")
import numpy as np

HID = 128
T, Q, S = 256, 64, 4
DI, DTXT = 2048, 768
NCORES = 8
TPC = T // NCORES               # 32 tasks per core
FEAT = DI + DTXT                # 2816
KT = FEAT // 128                # 22 contraction chunks
QROWS = TPC * Q                 # 2048 query rows per core
SROWS = TPC * S                 # 128 support rows per core
ROWS = QROWS + SROWS            # 2176
SCALE_INV = 1.0 / (np.sqrt(HID) + 1e-8)

_progs = {}  # cached compiled Bass programs, keyed by fast-path flag


def _build(fast):
    import concourse.bacc as bacc
    import concourse.tile as tile
    import concourse.mybir as mybir
    import concourse.bass as _b

    F32 = mybir.dt.float32
    F16 = mybir.dt.float16
    AFT = mybir.ActivationFunctionType

    nc = bacc.Bacc()
    x1_d = nc.declare_dram_parameter("x1", [128, KT * 640], F16, isOutput=False)
    x2_d = nc.declare_dram_parameter("x2", [128, KT * 1024], F16, isOutput=False)
    x3a_d = nc.declare_dram_parameter("x3a", [128, KT * 256], F16,
                                      isOutput=False)
    x3b_d = nc.declare_dram_parameter("x3b", [128, KT * 256], F16,
                                      isOutput=False)
    wc_d = nc.declare_dram_parameter("wc", [128, KT * 128], F16, isOutput=False)
    wu_d = nc.declare_dram_parameter("wu", [128, KT * 128], F16, isOutput=False)
    cst_d = nc.declare_dram_parameter("consts", [128, 772], F16, isOutput=False)
    row0_d = nc.declare_dram_parameter("row0", [1, 768], F16, isOutput=False)
    bias_d = nc.declare_dram_parameter("biases", [HID, 5], F32, isOutput=False)
    out_d = nc.declare_dram_parameter("logits", [TPC, Q, S], F32, isOutput=True)

    lp = nc.allow_low_precision(reason="fp16 streaming with f32 accumulation")
    lp.__enter__()

    with tile.TileContext(nc) as tc:
        with (
            tc.tile_pool(name="wts", bufs=1) as wts,
            tc.tile_pool(name="xg1p", bufs=1) as xg1p,
            tc.tile_pool(name="xg2p", bufs=1) as xg2p,
            tc.tile_pool(name="xg3p", bufs=1) as xg3p,
            tc.tile_pool(name="qfp", bufs=1) as qfp,
            tc.tile_pool(name="wk", bufs=2) as wk,
            tc.tile_pool(name="pst", bufs=1, space="PSUM") as pst,
        ):
            # ---- loads, in stream order ----
            wc_t = wts.tile([128, KT, 128], F16)
            nc.sync.dma_start(out=wc_t, in_=wc_d[:])
            wu_t = wts.tile([128, KT, 128], F16)
            nc.sync.dma_start(out=wu_t, in_=wu_d[:])
            xg1_t = xg1p.tile([128, KT, 640], F16)
            nc.sync.dma_start(out=xg1_t, in_=x1_d[:])
            cst_t = wts.tile([128, 772], F16)
            nc.sync.dma_start(out=cst_t, in_=cst_d[:])
            wq_t = cst_t[:, 0 * HID:1 * HID]
            wk_t = cst_t[:, 1 * HID:2 * HID]
            wv_t = cst_t[:, 2 * HID:3 * HID]
            wo_t = cst_t[:, 3 * HID:4 * HID]
            wov_t = cst_t[:, 4 * HID:5 * HID]
            red_ln = cst_t[:, 640:641]           # 1/128
            red_q = cst_t[:, 641:642]            # 1.0
            red_s = cst_t[:, 642:643]            # 0.01
            mask_t = cst_t[:, 644:772]           # [128, 128] 0/1 block mask
            row0_t = wts.tile([1, 768], F16)
            nc.sync.dma_start(out=row0_t, in_=row0_d[:])
            ones_r = row0_t[:, 0:128]            # lhsT [K=1, M=128] for bcast
            ones_row = row0_t[:, 0:640]          # all-ones rhs
            bvrow_t = row0_t[:, 640:768]
            bias_t = wts.tile([HID, 5], F32)
            nc.sync.dma_start(out=bias_t, in_=bias_d[:])
            bc_a, bq_a, bk_a, bv_a, bo_a = (bias_t[:, i:i + 1] for i in range(5))
            xg2_t = xg2p.tile([128, KT, 1024], F16)
            nc.sync.dma_start(out=xg2_t[:, 0:11, :], in_=x2_d[:, 0:11 * 1024])
            nc.sync.dma_start(out=xg2_t[:, 11:KT, :], in_=x2_d[:, 11 * 1024:])
            xg3a_t = xg3p.tile([128, KT, 256], F16)
            nc.sync.dma_start(out=xg3a_t, in_=x3a_d[:])
            xg3b_t = xg3p.tile([128, KT, 256], F16)
            nc.sync.dma_start(out=xg3b_t, in_=x3b_d[:])

            qf_t = qfp.tile([128, QROWS], F16)
            sf_t = qfp.tile([128, SROWS], F16)
            U_sb = qfp.tile([64, 2 * Q], F32)

            def rstd(sq_sb, cn, red):
                """1/sqrt(red . sq) broadcast to [128, cn] PSUM (f16 path)."""
                ss_ps = pst.tile([1, 512], F32, tag="ss", bufs=1)
                nc.tensor.matmul(ss_ps[:, :cn], red, sq_sb[:, :cn],
                                 start=True, stop=True)
                ir = wk.tile([1, 512], F32, tag="ir")
                nc.vector.reciprocal_approx_fast(out=ir[:, :cn],
                                                 in_=ss_ps[:, :cn])
                rr = wk.tile([1, 512], F16, tag="rr")
                nc.scalar.activation(out=rr[:, :cn], in_=ir[:, :cn],
                                     func=AFT.Sqrt, bias=0.0, scale=1.0)
                R_ps = pst.tile([128, 512], F32, tag="R", bufs=2)
                nc.tensor.matmul(R_ps[:, :cn], ones_r, rr[:, :cn],
                                 start=True, stop=True)
                return R_ps

            def score(t0, nt):
                """U_sb[q, 4t:4(t+nt)] = qf[:,64t:...]^T sf[:,4t:...] x nt."""
                U_ps = pst.tile([64, 32], F32, tag="sc", bufs=1)
                for j in range(nt):
                    t = t0 + j
                    nc.tensor.matmul(U_ps[0:64, 4 * j:4 * j + 4],
                                     qf_t[:, 64 * t:64 * t + 64],
                                     sf_t[:, 4 * t:4 * t + 4],
                                     start=True, stop=True)
                nc.vector.tensor_copy(
                    out=U_sb[0:64, 4 * t0:4 * (t0 + nt)],
                    in_=U_ps[0:64, 0:4 * nt])

            def query_tail_fast(u_ps, qf_off, cn):
                """qf = normalize(u); u = Wu x accumulated during streaming."""
                ff = wk.tile([128, 512], F16, tag="ff")
                nc.vector.tensor_copy(out=ff[:, :cn], in_=u_ps[:, :cn])
                sq = wk.tile([128, 512], F16, tag="sq")
                nc.vector.tensor_mul(out=sq[:, :cn], in0=ff[:, :cn],
                                     in1=ff[:, :cn])
                RN = rstd(sq, cn, red_q)
                nc.vector.tensor_mul(out=qf_t[:, qf_off:qf_off + cn],
                                     in0=ff[:, :cn], in1=RN[:, :cn])
                score(qf_off // 64, cn // 64)

            def query_tail_gen(c_ps, qf_off, cn):
                cf = wk.tile([128, 512], F16, tag="cf")
                nc.vector.tensor_scalar_add(out=cf[:, :cn], in0=c_ps[:, :cn],
                                            scalar1=bc_a)
                sq = wk.tile([128, 512], F16, tag="sq")
                nc.vector.tensor_mul(out=sq[:, :cn], in0=cf[:, :cn],
                                     in1=cf[:, :cn])
                R1 = rstd(sq, cn, red_ln)
                h = wk.tile([128, 512], F16, tag="h")
                nc.vector.tensor_mul(out=h[:, :cn], in0=cf[:, :cn],
                                     in1=R1[:, :cn])
                v_ps = pst.tile([128, 512], F32, tag="pp", bufs=2)
                nc.tensor.matmul(v_ps[:, :cn], wv_t, h[:, :cn],
                                 start=True, stop=True)
                vf = wk.tile([128, 512], F16, tag="vf")
                nc.vector.tensor_scalar_add(out=vf[:, :cn], in0=v_ps[:, :cn],
                                            scalar1=bv_a)
                sq2 = wk.tile([128, 512], F16, tag="sq")
                nc.vector.tensor_mul(out=sq2[:, :cn], in0=vf[:, :cn],
                                     in1=vf[:, :cn])
                R2 = rstd(sq2, cn, red_ln)
                z = wk.tile([128, 512], F16, tag="h")
                nc.vector.tensor_mul(out=z[:, :cn], in0=vf[:, :cn],
                                     in1=R2[:, :cn])
                o_ps = pst.tile([128, 512], F32, tag="pp", bufs=2)
                nc.tensor.matmul(o_ps[:, :cn], wo_t, z[:, :cn],
                                 start=True, stop=True)
                ff = wk.tile([128, 512], F16, tag="ff")
                nc.vector.tensor_scalar_add(out=ff[:, :cn], in0=o_ps[:, :cn],
                                            scalar1=bo_a)
                sq3 = wk.tile([128, 512], F16, tag="sq")
                nc.vector.tensor_mul(out=sq3[:, :cn], in0=ff[:, :cn],
                                     in1=ff[:, :cn])
                RN = rstd(sq3, cn, red_q)
                nc.vector.tensor_mul(out=qf_t[:, qf_off:qf_off + cn],
                                     in0=ff[:, :cn], in1=RN[:, :cn])
                score(qf_off // 64, cn // 64)

            query_tail = query_tail_fast if fast else query_tail_gen
            wq_stream = wu_t if fast else wc_t

            def support_tail(cS_ps):
                cfS = wk.tile([128, SROWS], F16, tag="cfS")
                if fast:
                    nc.vector.tensor_copy(out=cfS, in_=cS_ps[:, 0:SROWS])
                else:
                    nc.vector.tensor_scalar_add(out=cfS, in0=cS_ps[:, 0:SROWS],
                                                scalar1=bc_a)
                sqS = wk.tile([128, SROWS], F16, tag="sqS")
                nc.vector.tensor_mul(out=sqS, in0=cfS, in1=cfS)
                R1 = rstd(sqS, SROWS, red_ln)
                hs = wk.tile([128, SROWS], F16, tag="hs")
                nc.vector.tensor_mul(out=hs, in0=cfS, in1=R1[:, :SROWS])
                q_ps = pst.tile([128, 512], F32, tag="pp", bufs=2)
                nc.tensor.matmul(q_ps[:, :SROWS], wq_t, hs, start=True, stop=True)
                qTb = wk.tile([128, SROWS], F16, tag="qTb")
                nc.vector.tensor_scalar_add(out=qTb, in0=q_ps[:, :SROWS],
                                            scalar1=bq_a)
                k_ps = pst.tile([128, 512], F32, tag="pp", bufs=2)
                nc.tensor.matmul(k_ps[:, :SROWS], wk_t, hs, start=True, stop=True)
                kTb = wk.tile([128, SROWS], F16, tag="kTb")
                nc.vector.tensor_scalar_add(out=kTb, in0=k_ps[:, :SROWS],
                                            scalar1=bk_a)
                # vn = (Wv h)^T (+ 1 (x) bv in the general path) : [srows, hid]
                vn_ps = pst.tile([128, 512], F32, tag="pp", bufs=2)
                nc.tensor.matmul(vn_ps[:, :HID], hs, wv_t, start=True,
                                 stop=fast)
                if not fast:
                    nc.tensor.matmul(vn_ps[:, :HID], ones_row[:, 0:SROWS],
                                     bvrow_t, start=False, stop=True)
                vn_sb = wk.tile([SROWS, HID], F16, tag="vn")
                nc.vector.tensor_copy(out=vn_sb, in_=vn_ps[:, :HID])
                # sT[k_row, q_row] = k^T q ; unnormalized masked exp
                sT_ps = pst.tile([128, 512], F32, tag="pp", bufs=2)
                nc.tensor.matmul(sT_ps[:, :SROWS], kTb, qTb, start=True,
                                 stop=True)
                aT = wk.tile([SROWS, SROWS], F16, tag="aT")
                nc.scalar.activation(out=aT, in_=sT_ps[:, :SROWS], func=AFT.Exp,
                                     bias=0.0, scale=1.0)
                am = wk.tile([SROWS, SROWS], F16, tag="am")
                nc.vector.tensor_mul(out=am, in0=aT, in1=mask_t)
                ctx_ps = pst.tile([128, 512], F32, tag="pp", bufs=2)
                nc.tensor.matmul(ctx_ps[:, :SROWS], vn_sb, am, start=True,
                                 stop=True)
                cfx = wk.tile([128, SROWS], F16, tag="cfS")
                nc.vector.tensor_copy(out=cfx, in_=ctx_ps[:, :SROWS])
                if fast:
                    zx = cfx           # LN2 rstd cancels in the final normalize
                else:
                    sqx = wk.tile([128, SROWS], F16, tag="sqS")
                    nc.vector.tensor_mul(out=sqx, in0=cfx, in1=cfx)
                    R2 = rstd(sqx, SROWS, red_ln)
                    zx = wk.tile([128, SROWS], F16, tag="hs")
                    nc.vector.tensor_mul(out=zx, in0=cfx, in1=R2[:, :SROWS])
                o_ps = pst.tile([128, 512], F32, tag="pp", bufs=2)
                nc.tensor.matmul(o_ps[:, :SROWS], wo_t, zx, start=True, stop=True)
                ffx = wk.tile([128, SROWS], F16, tag="ffS")
                if fast:
                    nc.vector.tensor_copy(out=ffx, in_=o_ps[:, :SROWS])
                else:
                    nc.vector.tensor_scalar_add(out=ffx, in0=o_ps[:, :SROWS],
                                                scalar1=bo_a)
                sqf = wk.tile([128, SROWS], F16, tag="sqS")
                nc.vector.tensor_mul(out=sqf, in0=ffx, in1=ffx)
                RN = rstd(sqf, SROWS, red_s)   # = 10/||f||
                nc.vector.tensor_mul(out=sf_t[:], in0=ffx, in1=RN[:, :SROWS])

            def out_dma(t0, nt):
                dst = _b.AP(tensor=out_d[:].tensor,
                            offset=out_d[:].offset + 256 * t0,
                            ap=[[4, 64], [256, nt], [1, 4]])
                nc.sync.dma_start(
                    out=dst,
                    in_=U_sb[0:64, 4 * t0:4 * (t0 + nt)].rearrange(
                        "p (g b) -> p g b", b=4))

            # ---- group 1: support + q0 ----
            with tc.tile_pool(name="psA", bufs=1, space="PSUM") as psA:
                cS_ps = psA.tile([128, SROWS], F32)
                u0_ps = psA.tile([128, 512], F32)
                for k in range(KT):
                    nc.tensor.matmul(cS_ps[:], wc_t[:, k, :],
                                     xg1_t[:, k, 0:128],
                                     start=(k == 0), stop=(k == KT - 1))
                    nc.tensor.matmul(u0_ps[:], wq_stream[:, k, :],
                                     xg1_t[:, k, 128:640],
                                     start=(k == 0), stop=(k == KT - 1))
                support_tail(cS_ps)
                query_tail(u0_ps, 0, 512)

            # ---- group 2: q1 + q2 ----
            with tc.tile_pool(name="psB", bufs=1, space="PSUM") as psB:
                u1_ps = psB.tile([128, 512], F32)
                u2_ps = psB.tile([128, 512], F32)
                for k in range(KT):
                    nc.tensor.matmul(u1_ps[:], wq_stream[:, k, :],
                                     xg2_t[:, k, 0:512],
                                     start=(k == 0), stop=(k == KT - 1))
                    nc.tensor.matmul(u2_ps[:], wq_stream[:, k, :],
                                     xg2_t[:, k, 512:1024],
                                     start=(k == 0), stop=(k == KT - 1))
                query_tail(u1_ps, 512, 512)
                query_tail(u2_ps, 1024, 512)
                out_dma(0, 24)

            # ---- group 3: q3 in two 256-col halves ----
            with tc.tile_pool(name="psC", bufs=1, space="PSUM") as psC:
                ua_ps = psC.tile([128, 256], F32)
                ub_ps = psC.tile([128, 256], F32)
                for k in range(KT):
                    nc.tensor.matmul(ua_ps[:], wq_stream[:, k, :],
                                     xg3a_t[:, k, :],
                                     start=(k == 0), stop=(k == KT - 1))
                for k in range(KT):
                    nc.tensor.matmul(ub_ps[:], wq_stream[:, k, :],
                                     xg3b_t[:, k, :],
                                     start=(k == 0), stop=(k == KT - 1))
                query_tail(ua_ps, 1536, 256)
                out_dma(24, 4)
                query_tail(ub_ps, 1792, 256)
                out_dma(28, 4)

    lp.__exit__(None, None, None)
    nc.compile()
    return nc


def _host_prep(inputs):
    f32, f16 = np.float32, np.float16
    Wi, Wt = np.asarray(inputs["Wi"], f32), np.asarray(inputs["Wt"], f32)
    bi, bt = np.asarray(inputs["bi"], f32), np.asarray(inputs["bt"], f32)
    g1, b1 = np.asarray(inputs["g1"], f32), np.asarray(inputs["b1"], f32)
    g2, b2 = np.asarray(inputs["g2"], f32), np.asarray(inputs["b2"], f32)
    Wq, bq = np.asarray(inputs["Wq"], f32), np.asarray(inputs["bq"], f32)
    Wk, bk = np.asarray(inputs["Wk"], f32), np.asarray(inputs["bk"], f32)
    Wv, bv = np.asarray(inputs["Wv"], f32), np.asarray(inputs["bv"], f32)
    Wo, bo = np.asarray(inputs["Wo"], f32), np.asarray(inputs["bo"], f32)

    Wc = np.concatenate([Wi, Wt], axis=1)          # [128, 2816]
    bc = bi + bt
    Wc_c = Wc - Wc.mean(axis=0, keepdims=True)     # fold LN1 mean
    bc_c = bc - bc.mean()

    Wq_f = (Wq * g1[None, :]) * SCALE_INV
    bq_f = (bq + Wq @ b1) * SCALE_INV
    Wk_f = Wk * g1[None, :]
    bk_f = bk + Wk @ b1
    Wv_f = Wv * g1[None, :]
    bv_f = bv + Wv @ b1
    Wv_c = Wv_f - Wv_f.mean(axis=0, keepdims=True)  # fold LN2 mean
    bv_c = bv_f - bv_f.mean()
    Wo_f = Wo * g2[None, :]
    bo_f = bo + Wo @ b2
    Wov = Wo_f @ Wv_c                               # combined v+o projection
    Wu = Wov @ Wc_c                                 # full query-path fold

    fast = (np.abs(bc_c).max() < 1e-12 and np.abs(bv_c).max() < 1e-12
            and np.abs(bo_f).max() < 1e-12)

    blk = np.arange(SROWS) // S
    mask01 = (blk[:, None] == blk[None, :]).astype(f16)
    consts = np.zeros((128, 772), f16)
    consts[:, 0:640] = np.concatenate(
        [Wq_f.T, Wk_f.T, Wv_c.T, Wo_f.T, Wov.T], axis=1).astype(f16)
    consts[:, 640] = f16(1.0 / HID)
    consts[:, 641] = f16(1.0)
    consts[:, 642] = f16(0.01)
    consts[:, 644:772] = mask01
    row0 = np.zeros((1, 768), f16)
    row0[0, 0:640] = 1.0
    row0[0, 640:768] = bv_c.astype(f16)

    def pack_kmajor(a):   # [feat, cols] -> [128, KT*cols] (p, k, c)
        cols = a.shape[1]
        return np.ascontiguousarray(
            a.reshape(KT, 128, cols).transpose(1, 0, 2).reshape(128, -1)
        )

    common = {
        "wc": pack_kmajor(Wc_c.T.astype(f16)),
        "wu": pack_kmajor(Wu.T.astype(f16)),
        "consts": consts,
        "row0": row0,
        "biases": np.ascontiguousarray(
            np.stack([bc_c, bq_f, bk_f, bv_c, bo_f], axis=1)),
    }

    si = np.asarray(inputs["support_images"], f32)
    st = np.asarray(inputs["support_texts"], f32)
    qi = np.asarray(inputs["query_images"], f32)
    qt = np.asarray(inputs["query_texts"], f32)

    in_maps = []
    for m in range(NCORES):
        ts = slice(m * TPC, (m + 1) * TPC)
        Xq = np.concatenate([qi[ts].reshape(QROWS, DI),
                             qt[ts].reshape(QROWS, DTXT)], axis=1)
        Xs = np.concatenate([si[ts].reshape(SROWS, DI),
                             st[ts].reshape(SROWS, DTXT)], axis=1)
        xT = np.concatenate([Xs, Xq], axis=0).T.astype(f16)  # [2816, 2176]
        in_maps.append({
            "x1": pack_kmajor(xT[:, 0:640]),
            "x2": pack_kmajor(xT[:, 640:1664]),
            "x3a": pack_kmajor(xT[:, 1664:1920]),
            "x3b": pack_kmajor(xT[:, 1920:2176]),
            **common,
        })
    return in_maps, fast


def _run(in_maps, fast, trace=False, **kw):
    from concourse.bass_utils import run_bass_kernel_spmd
    if fast not in _progs:
        _progs[fast] = _build(fast)
    return run_bass_kernel_spmd(_progs[fast], in_maps, list(range(NCORES)),
                                trace=trace, **kw)


def kernel(**inputs) -> np.ndarray:
    in_maps, fast = _host_prep(inputs)
    res = _run(in_maps, fast)
    return np.concatenate([res.results[m]["logits"] for m in range(NCORES)],
                          axis=0)


# revision 10
# speedup vs baseline: 2.8992x; 1.0375x over previous
"""Trainium2 Bass kernel for nn_MetaLearner (meta-learning attention + cosine
prototype scoring), data-parallel over tasks on 8 NeuronCores.

Math (per task):
  c   = [img, txt] @ Wc.T + bc                (Wc = concat(Wi, Wt))
  h   = LN1(c);  q,k,v = h @ W{q,k,v}.T + b   (queries: seqlen=1 -> ctx = v)
  ctx = softmax(q k^T / sqrt(128)) v          (support: seqlen=4)
  f   = LN2(ctx) @ Wo.T + bo
  logits[t,q,c] = 10 * cos(qf[t,q], sf[t,c])

Key tricks:
  - LN gains/biases and mean-subtractions folded into weights on host.
  - Softmax denominator (and max-subtraction) dropped: LN2's rstd cancels
    any positive per-column scale of ctx; mean-centering folded into Wv.
  - When bc/bv/bo fold to zero (true for the reference initialization),
    every per-column scale cancels through the final cosine normalize, so
    the whole query path collapses to qf = normalize(Wu x) with
    Wu = (Wo Wv_c) Wc_c folded on host -- queries never materialize c, h,
    v, or f; the projection accumulates directly during streaming.
    Support keeps LN1 (softmax is scale-sensitive) but skips LN2's rstd.
    A general fallback path keeps the full math.
  - rstd = Sqrt(reciprocal_approx_fast(scale * colsum(x^2))): the scale
    rides in the reduction weights, DVE does the reciprocal, ACT only ever
    evaluates Sqrt (plus one Exp for softmax) -> ~2 table switches total.
  - No transposes: attention scores computed pre-transposed (sT = kT^T qT)
    and v computed pre-transposed (vn = h^T WvT) by operand swapping.
  - All PE operands fp16 (1 cyc/row); f32 accumulation in PSUM.
  - Inputs streamed as fp16, host-packed per column-group so every stream
    DMA is one contiguous 1.4-5.6 MB transfer. The last group is only 256
    columns so the final (unhidable) tail chain is short.
On-chip layout is "transposed" throughout: activations are [hid, rows].
"""
import sys
sys.path.insert(0, "/opt/trn_rl_repo")
import numpy as np

HID = 128
T, Q, S = 256, 64, 4
DI, DTXT = 2048, 768
NCORES = 8
TPC = T // NCORES               # 32 tasks per core
FEAT = DI + DTXT                # 2816
KT = FEAT // 128                # 22 contraction chunks
QROWS = TPC * Q                 # 2048 query rows per core
SROWS = TPC * S                 # 128 support rows per core
ROWS = QROWS + SROWS            # 2176
SCALE_INV = 1.0 / (np.sqrt(HID) + 1e-8)

_progs = {}  # cached compiled Bass programs, keyed by fast-path flag


def _build(fast):
    import concourse.bacc as bacc
    import concourse.tile as tile
    import concourse.mybir as mybir
    import concourse.bass as _b

    F32 = mybir.dt.float32
    F16 = mybir.dt.float16
    AFT = mybir.ActivationFunctionType

    nc = bacc.Bacc()
    x1_d = nc.declare_dram_parameter("x1", [128, KT * 640], F16, isOutput=False)
    x2_d = nc.declare_dram_parameter("x2", [128, KT * 1024], F16, isOutput=False)
    x3a_d = nc.declare_dram_parameter("x3a", [128, KT * 256], F16,
                                      isOutput=False)
    x3b_d = nc.declare_dram_parameter("x3b", [128, KT * 256], F16,
                                      isOutput=False)
    wc_d = nc.declare_dram_parameter("wc", [128, KT * 128], F16, isOutput=False)
    wu_d = nc.declare_dram_parameter("wu", [128, KT * 128], F16, isOutput=False)
    cst_d = nc.declare_dram_parameter("consts", [128, 772], F16, isOutput=False)
    row0_d = nc.declare_dram_parameter("row0", [1, 768], F16, isOutput=False)
    bias_d = nc.declare_dram_parameter("biases", [HID, 5], F32, isOutput=False)
    out_d = nc.declare_dram_parameter("logits", [TPC, Q, S], F32, isOutput=True)

    lp = nc.allow_low_precision(reason="fp16 streaming with f32 accumulation")
    lp.__enter__()

    with tile.TileContext(nc) as tc:
        with (
            tc.tile_pool(name="wts", bufs=1) as wts,
            tc.tile_pool(name="xg1p", bufs=1) as xg1p,
            tc.tile_pool(name="xg2p", bufs=1) as xg2p,
            tc.tile_pool(name="xg3p", bufs=1) as xg3p,
            tc.tile_pool(name="qfp", bufs=1) as qfp,
            tc.tile_pool(name="wk", bufs=2) as wk,
            tc.tile_pool(name="pst", bufs=1, space="PSUM") as pst,
        ):
            # ---- loads, in stream order ----
            wc_t = wts.tile([128, KT, 128], F16)
            nc.sync.dma_start(out=wc_t, in_=wc_d[:])
            wu_t = wts.tile([128, KT, 128], F16)
            nc.sync.dma_start(out=wu_t, in_=wu_d[:])
            xg1_t = xg1p.tile([128, KT, 640], F16)
            nc.sync.dma_start(out=xg1_t[:, 0:11, :], in_=x1_d[:, 0:11 * 640])
            nc.sync.dma_start(out=xg1_t[:, 11:KT, :], in_=x1_d[:, 11 * 640:])
            cst_t = wts.tile([128, 772], F16)
            nc.sync.dma_start(out=cst_t, in_=cst_d[:])
            wq_t = cst_t[:, 0 * HID:1 * HID]
            wk_t = cst_t[:, 1 * HID:2 * HID]
            wv_t = cst_t[:, 2 * HID:3 * HID]
            wo_t = cst_t[:, 3 * HID:4 * HID]
            wov_t = cst_t[:, 4 * HID:5 * HID]
            red_ln = cst_t[:, 640:641]           # 1/128
            red_q = cst_t[:, 641:642]            # 1.0
            red_s = cst_t[:, 642:643]            # 0.01
            mask_t = cst_t[:, 644:772]           # [128, 128] 0/1 block mask
            row0_t = wts.tile([1, 768], F16)
            nc.sync.dma_start(out=row0_t, in_=row0_d[:])
            ones_r = row0_t[:, 0:128]            # lhsT [K=1, M=128] for bcast
            ones_row = row0_t[:, 0:640]          # all-ones rhs
            bvrow_t = row0_t[:, 640:768]
            bias_t = wts.tile([HID, 5], F32)
            nc.sync.dma_start(out=bias_t, in_=bias_d[:])
            bc_a, bq_a, bk_a, bv_a, bo_a = (bias_t[:, i:i + 1] for i in range(5))
            xg2_t = xg2p.tile([128, KT, 1024], F16)
            for (a, b) in ((0, 6), (6, 11), (11, 17), (17, KT)):
                nc.sync.dma_start(out=xg2_t[:, a:b, :],
                                  in_=x2_d[:, a * 1024:b * 1024])
            xg3a_t = xg3p.tile([128, KT, 256], F16)
            nc.sync.dma_start(out=xg3a_t, in_=x3a_d[:])
            xg3b_t = xg3p.tile([128, KT, 256], F16)
            nc.sync.dma_start(out=xg3b_t, in_=x3b_d[:])

            qf_t = qfp.tile([128, QROWS], F16)
            sf_t = qfp.tile([128, SROWS], F16)
            U_sb = qfp.tile([64, 2 * Q], F32)

            def rstd(sq_sb, cn, red):
                """1/sqrt(red . sq) broadcast to [128, cn] PSUM (f16 path)."""
                ss_ps = pst.tile([1, 512], F32, tag="ss", bufs=1)
                nc.tensor.matmul(ss_ps[:, :cn], red, sq_sb[:, :cn],
                                 start=True, stop=True)
                ir = wk.tile([1, 512], F32, tag="ir")
                nc.vector.reciprocal_approx_fast(out=ir[:, :cn],
                                                 in_=ss_ps[:, :cn])
                rr = wk.tile([1, 512], F16, tag="rr")
                nc.scalar.activation(out=rr[:, :cn], in_=ir[:, :cn],
                                     func=AFT.Sqrt, bias=0.0, scale=1.0)
                R_ps = pst.tile([128, 512], F32, tag="R", bufs=2)
                nc.tensor.matmul(R_ps[:, :cn], ones_r, rr[:, :cn],
                                 start=True, stop=True)
                return R_ps

            def score(t0, nt):
                """U_sb[q, 4t:4(t+nt)] = qf[:,64t:...]^T sf[:,4t:...] x nt."""
                U_ps = pst.tile([64, 32], F32, tag="sc", bufs=1)
                for j in range(nt):
                    t = t0 + j
                    nc.tensor.matmul(U_ps[0:64, 4 * j:4 * j + 4],
                                     qf_t[:, 64 * t:64 * t + 64],
                                     sf_t[:, 4 * t:4 * t + 4],
                                     start=True, stop=True)
                nc.vector.tensor_copy(
                    out=U_sb[0:64, 4 * t0:4 * (t0 + nt)],
                    in_=U_ps[0:64, 0:4 * nt])

            def query_tail_fast(u_ps, qf_off, cn):
                """qf = normalize(u); u = Wu x accumulated during streaming."""
                ff = wk.tile([128, 512], F16, tag="ff")
                nc.vector.tensor_copy(out=ff[:, :cn], in_=u_ps[:, :cn])
                sq = wk.tile([128, 512], F16, tag="sq")
                nc.vector.tensor_mul(out=sq[:, :cn], in0=ff[:, :cn],
                                     in1=ff[:, :cn])
                RN = rstd(sq, cn, red_q)
                nc.vector.tensor_mul(out=qf_t[:, qf_off:qf_off + cn],
                                     in0=ff[:, :cn], in1=RN[:, :cn])
                score(qf_off // 64, cn // 64)

            def query_tail_gen(c_ps, qf_off, cn):
                cf = wk.tile([128, 512], F16, tag="cf")
                nc.vector.tensor_scalar_add(out=cf[:, :cn], in0=c_ps[:, :cn],
                                            scalar1=bc_a)
                sq = wk.tile([128, 512], F16, tag="sq")
                nc.vector.tensor_mul(out=sq[:, :cn], in0=cf[:, :cn],
                                     in1=cf[:, :cn])
                R1 = rstd(sq, cn, red_ln)
                h = wk.tile([128, 512], F16, tag="h")
                nc.vector.tensor_mul(out=h[:, :cn], in0=cf[:, :cn],
                                     in1=R1[:, :cn])
                v_ps = pst.tile([128, 512], F32, tag="pp", bufs=2)
                nc.tensor.matmul(v_ps[:, :cn], wv_t, h[:, :cn],
                                 start=True, stop=True)
                vf = wk.tile([128, 512], F16, tag="vf")
                nc.vector.tensor_scalar_add(out=vf[:, :cn], in0=v_ps[:, :cn],
                                            scalar1=bv_a)
                sq2 = wk.tile([128, 512], F16, tag="sq")
                nc.vector.tensor_mul(out=sq2[:, :cn], in0=vf[:, :cn],
                                     in1=vf[:, :cn])
                R2 = rstd(sq2, cn, red_ln)
                z = wk.tile([128, 512], F16, tag="h")
                nc.vector.tensor_mul(out=z[:, :cn], in0=vf[:, :cn],
                                     in1=R2[:, :cn])
                o_ps = pst.tile([128, 512], F32, tag="pp", bufs=2)
                nc.tensor.matmul(o_ps[:, :cn], wo_t, z[:, :cn],
                                 start=True, stop=True)
                ff = wk.tile([128, 512], F16, tag="ff")
                nc.vector.tensor_scalar_add(out=ff[:, :cn], in0=o_ps[:, :cn],
                                            scalar1=bo_a)
                sq3 = wk.tile([128, 512], F16, tag="sq")
                nc.vector.tensor_mul(out=sq3[:, :cn], in0=ff[:, :cn],
                                     in1=ff[:, :cn])
                RN = rstd(sq3, cn, red_q)
                nc.vector.tensor_mul(out=qf_t[:, qf_off:qf_off + cn],
                                     in0=ff[:, :cn], in1=RN[:, :cn])
                score(qf_off // 64, cn // 64)

            query_tail = query_tail_fast if fast else query_tail_gen
            wq_stream = wu_t if fast else wc_t

            def support_tail(cS_ps):
                cfS = wk.tile([128, SROWS], F16, tag="cfS")
                if fast:
                    nc.vector.tensor_copy(out=cfS, in_=cS_ps[:, 0:SROWS])
                else:
                    nc.vector.tensor_scalar_add(out=cfS, in0=cS_ps[:, 0:SROWS],
                                                scalar1=bc_a)
                sqS = wk.tile([128, SROWS], F16, tag="sqS")
                nc.vector.tensor_mul(out=sqS, in0=cfS, in1=cfS)
                R1 = rstd(sqS, SROWS, red_ln)
                hs = wk.tile([128, SROWS], F16, tag="hs")
                nc.vector.tensor_mul(out=hs, in0=cfS, in1=R1[:, :SROWS])
                q_ps = pst.tile([128, 512], F32, tag="pp", bufs=2)
                nc.tensor.matmul(q_ps[:, :SROWS], wq_t, hs, start=True, stop=True)
                qTb = wk.tile([128, SROWS], F16, tag="qTb")
                nc.vector.tensor_scalar_add(out=qTb, in0=q_ps[:, :SROWS],
                                            scalar1=bq_a)
                k_ps = pst.tile([128, 512], F32, tag="pp", bufs=2)
                nc.tensor.matmul(k_ps[:, :SROWS], wk_t, hs, start=True, stop=True)
                kTb = wk.tile([128, SROWS], F16, tag="kTb")
                nc.vector.tensor_scalar_add(out=kTb, in0=k_ps[:, :SROWS],
                                            scalar1=bk_a)
                # vn = (Wv h)^T (+ 1 (x) bv in the general path) : [srows, hid]
                vn_ps = pst.tile([128, 512], F32, tag="pp", bufs=2)
                nc.tensor.matmul(vn_ps[:, :HID], hs, wv_t, start=True,
                                 stop=fast)
                if not fast:
                    nc.tensor.matmul(vn_ps[:, :HID], ones_row[:, 0:SROWS],
                                     bvrow_t, start=False, stop=True)
                vn_sb = wk.tile([SROWS, HID], F16, tag="vn")
                nc.vector.tensor_copy(out=vn_sb, in_=vn_ps[:, :HID])
                # sT[k_row, q_row] = k^T q ; unnormalized masked exp
                sT_ps = pst.tile([128, 512], F32, tag="pp", bufs=2)
                nc.tensor.matmul(sT_ps[:, :SROWS], kTb, qTb, start=True,
                                 stop=True)
                aT = wk.tile([SROWS, SROWS], F16, tag="aT")
                nc.scalar.activation(out=aT, in_=sT_ps[:, :SROWS], func=AFT.Exp,
                                     bias=0.0, scale=1.0)
                am = wk.tile([SROWS, SROWS], F16, tag="am")
                nc.vector.tensor_mul(out=am, in0=aT, in1=mask_t)
                ctx_ps = pst.tile([128, 512], F32, tag="pp", bufs=2)
                nc.tensor.matmul(ctx_ps[:, :SROWS], vn_sb, am, start=True,
                                 stop=True)
                cfx = wk.tile([128, SROWS], F16, tag="cfS")
                nc.vector.tensor_copy(out=cfx, in_=ctx_ps[:, :SROWS])
                if fast:
                    zx = cfx           # LN2 rstd cancels in the final normalize
                else:
                    sqx = wk.tile([128, SROWS], F16, tag="sqS")
                    nc.vector.tensor_mul(out=sqx, in0=cfx, in1=cfx)
                    R2 = rstd(sqx, SROWS, red_ln)
                    zx = wk.tile([128, SROWS], F16, tag="hs")
                    nc.vector.tensor_mul(out=zx, in0=cfx, in1=R2[:, :SROWS])
                o_ps = pst.tile([128, 512], F32, tag="pp", bufs=2)
                nc.tensor.matmul(o_ps[:, :SROWS], wo_t, zx, start=True, stop=True)
                ffx = wk.tile([128, SROWS], F16, tag="ffS")
                if fast:
                    nc.vector.tensor_copy(out=ffx, in_=o_ps[:, :SROWS])
                else:
                    nc.vector.tensor_scalar_add(out=ffx, in0=o_ps[:, :SROWS],
                                                scalar1=bo_a)
                sqf = wk.tile([128, SROWS], F16, tag="sqS")
                nc.vector.tensor_mul(out=sqf, in0=ffx, in1=ffx)
                RN = rstd(sqf, SROWS, red_s)   # = 10/||f||
                nc.vector.tensor_mul(out=sf_t[:], in0=ffx, in1=RN[:, :SROWS])

            def out_dma(t0, nt):
                dst = _b.AP(tensor=out_d[:].tensor,
                            offset=out_d[:].offset + 256 * t0,
                            ap=[[4, 64], [256, nt], [1, 4]])
                nc.sync.dma_start(
                    out=dst,
                    in_=U_sb[0:64, 4 * t0:4 * (t0 + nt)].rearrange(
                        "p (g b) -> p g b", b=4))

            # ---- group 1: support + q0 ----
            with tc.tile_pool(name="psA", bufs=1, space="PSUM") as psA:
                cS_ps = psA.tile([128, SROWS], F32)
                u0_ps = psA.tile([128, 512], F32)
                for k in range(KT):
                    nc.tensor.matmul(cS_ps[:], wc_t[:, k, :],
                                     xg1_t[:, k, 0:128],
                                     start=(k == 0), stop=(k == KT - 1))
                    nc.tensor.matmul(u0_ps[:], wq_stream[:, k, :],
                                     xg1_t[:, k, 128:640],
                                     start=(k == 0), stop=(k == KT - 1))
                support_tail(cS_ps)
                query_tail(u0_ps, 0, 512)

            # ---- group 2: q1 + q2 ----
            with tc.tile_pool(name="psB", bufs=1, space="PSUM") as psB:
                u1_ps = psB.tile([128, 512], F32)
                u2_ps = psB.tile([128, 512], F32)
                for k in range(KT):
                    nc.tensor.matmul(u1_ps[:], wq_stream[:, k, :],
                                     xg2_t[:, k, 0:512],
                                     start=(k == 0), stop=(k == KT - 1))
                    nc.tensor.matmul(u2_ps[:], wq_stream[:, k, :],
                                     xg2_t[:, k, 512:1024],
                                     start=(k == 0), stop=(k == KT - 1))
                query_tail(u1_ps, 512, 512)
                query_tail(u2_ps, 1024, 512)
                out_dma(0, 24)

            # ---- group 3: q3 in two 256-col halves ----
            with tc.tile_pool(name="psC", bufs=1, space="PSUM") as psC:
                ua_ps = psC.tile([128, 256], F32)
                ub_ps = psC.tile([128, 256], F32)
                for k in range(KT):
                    nc.tensor.matmul(ua_ps[:], wq_stream[:, k, :],
                                     xg3a_t[:, k, :],
                                     start=(k == 0), stop=(k == KT - 1))
                for k in range(KT):
                    nc.tensor.matmul(ub_ps[:], wq_stream[:, k, :],
                                     xg3b_t[:, k, :],
                                     start=(k == 0), stop=(k == KT - 1))
                query_tail(ua_ps, 1536, 256)
                out_dma(24, 4)
                query_tail(ub_ps, 1792, 256)
                out_dma(28, 4)

    lp.__exit__(None, None, None)
    nc.compile()
    return nc


def _host_prep(inputs):
    f32, f16 = np.float32, np.float16
    Wi, Wt = np.asarray(inputs["Wi"], f32), np.asarray(inputs["Wt"], f32)
    bi, bt = np.asarray(inputs["bi"], f32), np.asarray(inputs["bt"], f32)
    g1, b1 = np.asarray(inputs["g1"], f32), np.asarray(inputs["b1"], f32)
    g2, b2 = np.asarray(inputs["g2"], f32), np.asarray(inputs["b2"], f32)
    Wq, bq = np.asarray(inputs["Wq"], f32), np.asarray(inputs["bq"], f32)
    Wk, bk = np.asarray(inputs["Wk"], f32), np.asarray(inputs["bk"], f32)
    Wv, bv = np.asarray(inputs["Wv"], f32), np.asarray(inputs["bv"], f32)
    Wo, bo = np.asarray(inputs["Wo"], f32), np.asarray(inputs["bo"], f32)

    Wc = np.concatenate([Wi, Wt], axis=1)          # [128, 2816]
    bc = bi + bt
    Wc_c = Wc - Wc.mean(axis=0, keepdims=True)     # fold LN1 mean
    bc_c = bc - bc.mean()

    Wq_f = (Wq * g1[None, :]) * SCALE_INV
    bq_f = (bq + Wq @ b1) * SCALE_INV
    Wk_f = Wk * g1[None, :]
    bk_f = bk + Wk @ b1
    Wv_f = Wv * g1[None, :]
    bv_f = bv + Wv @ b1
    Wv_c = Wv_f - Wv_f.mean(axis=0, keepdims=True)  # fold LN2 mean
    bv_c = bv_f - bv_f.mean()
    Wo_f = Wo * g2[None, :]
    bo_f = bo + Wo @ b2
    Wov = Wo_f @ Wv_c                               # combined v+o projection
    Wu = Wov @ Wc_c                                 # full query-path fold

    fast = (np.abs(bc_c).max() < 1e-12 and np.abs(bv_c).max() < 1e-12
            and np.abs(bo_f).max() < 1e-12)

    blk = np.arange(SROWS) // S
    mask01 = (blk[:, None] == blk[None, :]).astype(f16)
    consts = np.zeros((128, 772), f16)
    consts[:, 0:640] = np.concatenate(
        [Wq_f.T, Wk_f.T, Wv_c.T, Wo_f.T, Wov.T], axis=1).astype(f16)
    consts[:, 640] = f16(1.0 / HID)
    consts[:, 641] = f16(1.0)
    consts[:, 642] = f16(0.01)
    consts[:, 644:772] = mask01
    row0 = np.zeros((1, 768), f16)
    row0[0, 0:640] = 1.0
    row0[0, 640:768] = bv_c.astype(f16)

    def pack_kmajor(a):   # [feat, cols] -> [128, KT*cols] (p, k, c)
        cols = a.shape[1]
        return np.ascontiguousarray(
            a.reshape(KT, 128, cols).transpose(1, 0, 2).reshape(128, -1)
        )

    common = {
        "wc": pack_kmajor(Wc_c.T.astype(f16)),
        "wu": pack_kmajor(Wu.T.astype(f16)),
        "consts": consts,
        "row0": row0,
        "biases": np.ascontiguousarray(
            np.stack([bc_c, bq_f, bk_f, bv_c, bo_f], axis=1)),
    }

    si = np.asarray(inputs["support_images"], f32)
    st = np.asarray(inputs["support_texts"], f32)
    qi = np.asarray(inputs["query_images"], f32)
    qt = np.asarray(inputs["query_texts"], f32)

    in_maps = []
    for m in range(NCORES):
        ts = slice(m * TPC, (m + 1) * TPC)
        Xq = np.concatenate([qi[ts].reshape(QROWS, DI),
                             qt[ts].reshape(QROWS, DTXT)], axis=1)
        Xs = np.concatenate([si[ts].reshape(SROWS, DI),
                             st[ts].reshape(SROWS, DTXT)], axis=1)
        xT = np.concatenate([Xs, Xq], axis=0).T.astype(f16)  # [2816, 2176]
        in_maps.append({
            "x1": pack_kmajor(xT[:, 0:640]),
            "x2": pack_kmajor(xT[:, 640:1664]),
            "x3a": pack_kmajor(xT[:, 1664:1920]),
            "x3b": pack_kmajor(xT[:, 1920:2176]),
            **common,
        })
    return in_maps, fast


def _run(in_maps, fast, trace=False, **kw):
    from concourse.bass_utils import run_bass_kernel_spmd
    if fast not in _progs:
        _progs[fast] = _build(fast)
    return run_bass_kernel_spmd(_progs[fast], in_maps, list(range(NCORES)),
                                trace=trace, **kw)


def kernel(**inputs) -> np.ndarray:
    in_maps, fast = _host_prep(inputs)
    res = _run(in_maps, fast)
    return np.concatenate([res.results[m]["logits"] for m in range(NCORES)],
                          axis=0)


# revision 13
# speedup vs baseline: 2.9855x; 1.0298x over previous
"""Trainium2 Bass kernel for nn_MetaLearner (meta-learning attention + cosine
prototype scoring), data-parallel over tasks on 8 NeuronCores.

Math (per task):
  c   = [img, txt] @ Wc.T + bc                (Wc = concat(Wi, Wt))
  h   = LN1(c);  q,k,v = h @ W{q,k,v}.T + b   (queries: seqlen=1 -> ctx = v)
  ctx = softmax(q k^T / sqrt(128)) v          (support: seqlen=4)
  f   = LN2(ctx) @ Wo.T + bo
  logits[t,q,c] = 10 * cos(qf[t,q], sf[t,c])

Key tricks:
  - LN gains/biases and mean-subtractions folded into weights on host.
  - Softmax denominator (and max-subtraction) dropped: LN2's rstd cancels
    any positive per-column scale of ctx; mean-centering folded into Wv.
  - When bc/bv/bo fold to zero (true for the reference initialization),
    every per-column scale cancels through the final cosine normalize, so
    the whole query path collapses to qf = normalize(Wu x) with
    Wu = (Wo Wv_c) Wc_c folded on host -- queries never materialize c, h,
    v, or f; the projection accumulates directly during streaming.
    Support keeps LN1 (softmax is scale-sensitive) but skips LN2's rstd.
    A general fallback path keeps the full math.
  - rstd = Sqrt(reciprocal_approx_fast(scale * colsum(x^2))): the scale
    rides in the reduction weights, DVE does the reciprocal, ACT only ever
    evaluates Sqrt (plus one Exp for softmax) -> ~2 table switches total.
  - No transposes: attention scores computed pre-transposed (sT = kT^T qT)
    and v computed pre-transposed (vn = h^T WvT) by operand swapping.
  - All PE operands fp16 (1 cyc/row); f32 accumulation in PSUM.
  - Inputs streamed as fp16, host-packed per column-group so every stream
    DMA is one contiguous 1.4-5.6 MB transfer. The last group is only 256
    columns so the final (unhidable) tail chain is short.
On-chip layout is "transposed" throughout: activations are [hid, rows].
"""
import sys
sys.path.insert(0, "/opt/trn_rl_repo")
import numpy as np

HID = 128
T, Q, S = 256, 64, 4
DI, DTXT = 2048, 768
NCORES = 8
TPC = T // NCORES               # 32 tasks per core
FEAT = DI + DTXT                # 2816
KT = FEAT // 128                # 22 contraction chunks
QROWS = TPC * Q                 # 2048 query rows per core
SROWS = TPC * S                 # 128 support rows per core
ROWS = QROWS + SROWS            # 2176
SCALE_INV = 1.0 / (np.sqrt(HID) + 1e-8)

_progs = {}  # cached compiled Bass programs, keyed by fast-path flag


def _build(fast):
    import concourse.bacc as bacc
    import concourse.tile as tile
    import concourse.mybir as mybir
    import concourse.bass as _b

    F32 = mybir.dt.float32
    F16 = mybir.dt.float16
    AFT = mybir.ActivationFunctionType

    nc = bacc.Bacc()
    x1_d = nc.declare_dram_parameter("x1", [128, KT * 640], F16, isOutput=False)
    x2_d = nc.declare_dram_parameter("x2", [128, KT * 1024], F16, isOutput=False)
    x3a_d = nc.declare_dram_parameter("x3a", [128, KT * 320], F16,
                                      isOutput=False)
    x3b_d = nc.declare_dram_parameter("x3b", [128, KT * 192], F16,
                                      isOutput=False)
    wcwu_d = nc.declare_dram_parameter("wcwu", [128, KT * 256], F16,
                                       isOutput=False)
    cst_d = nc.declare_dram_parameter("consts", [128, 772], F16, isOutput=False)
    row0_d = nc.declare_dram_parameter("row0", [1, 768], F16, isOutput=False)
    bias_d = (None if fast else
              nc.declare_dram_parameter("biases", [HID, 5], F32,
                                        isOutput=False))
    out_d = nc.declare_dram_parameter("logits", [TPC, Q, S], F32, isOutput=True)

    lp = nc.allow_low_precision(reason="fp16 streaming with f32 accumulation")
    lp.__enter__()

    with tile.TileContext(nc) as tc:
        with (
            tc.tile_pool(name="wts", bufs=1) as wts,
            tc.tile_pool(name="xg1p", bufs=1) as xg1p,
            tc.tile_pool(name="xg2p", bufs=1) as xg2p,
            tc.tile_pool(name="xg3p", bufs=1) as xg3p,
            tc.tile_pool(name="qfp", bufs=1) as qfp,
            tc.tile_pool(name="wk", bufs=2) as wk,
            tc.tile_pool(name="pst", bufs=1, space="PSUM") as pst,
        ):
            # ---- loads, in stream order ----
            wcwu_t = wts.tile([128, KT, 256], F16)
            nc.sync.dma_start(out=wcwu_t, in_=wcwu_d[:])
            xg1_t = xg1p.tile([128, KT, 640], F16)
            nc.sync.dma_start(out=xg1_t[:, 0:11, :], in_=x1_d[:, 0:11 * 640])
            nc.sync.dma_start(out=xg1_t[:, 11:KT, :], in_=x1_d[:, 11 * 640:])
            cst_t = wts.tile([128, 772], F16)
            nc.sync.dma_start(out=cst_t, in_=cst_d[:])
            wq_t = cst_t[:, 0 * HID:1 * HID]
            wk_t = cst_t[:, 1 * HID:2 * HID]
            wv_t = cst_t[:, 2 * HID:3 * HID]
            wo_t = cst_t[:, 3 * HID:4 * HID]
            wov_t = cst_t[:, 4 * HID:5 * HID]
            red_ln = cst_t[:, 640:641]           # 1/128
            red_q = cst_t[:, 641:642]            # 1.0
            red_s = cst_t[:, 642:643]            # 0.01
            mask_t = cst_t[:, 644:772]           # [128, 128] 0/1 block mask
            row0_t = wts.tile([1, 768], F16)
            nc.sync.dma_start(out=row0_t, in_=row0_d[:])
            ones_r = row0_t[:, 0:128]            # lhsT [K=1, M=128] for bcast
            ones_row = row0_t[:, 0:640]          # all-ones rhs
            bvrow_t = row0_t[:, 640:768]
            if not fast:
                bias_t = wts.tile([HID, 5], F32)
                nc.sync.dma_start(out=bias_t, in_=bias_d[:])
                bc_a, bq_a, bk_a, bv_a, bo_a = (
                    bias_t[:, i:i + 1] for i in range(5))
            else:
                bc_a = bq_a = bk_a = bv_a = bo_a = None
            xg2_t = xg2p.tile([128, KT, 1024], F16)
            for (a, b) in ((0, 6), (6, 11), (11, 17), (17, KT)):
                nc.sync.dma_start(out=xg2_t[:, a:b, :],
                                  in_=x2_d[:, a * 1024:b * 1024])
            xg3a_t = xg3p.tile([128, KT, 320], F16)
            nc.sync.dma_start(out=xg3a_t[:, 0:11, :], in_=x3a_d[:, 0:11 * 320])
            nc.sync.dma_start(out=xg3a_t[:, 11:KT, :], in_=x3a_d[:, 11 * 320:])
            xg3b_t = xg3p.tile([128, KT, 192], F16)
            nc.sync.dma_start(out=xg3b_t, in_=x3b_d[:])

            qf_t = qfp.tile([128, QROWS], F16)
            sf_t = qfp.tile([128, SROWS], F16)
            U_sb = qfp.tile([64, 2 * Q], F32)

            def rstd(sq_sb, cn, red):
                """1/sqrt(red . sq) broadcast to [128, cn] PSUM (f16 path)."""
                ss_ps = pst.tile([1, 512], F32, tag="ss", bufs=2)
                nc.tensor.matmul(ss_ps[:, :cn], red, sq_sb[:, :cn],
                                 start=True, stop=True)
                ir = wk.tile([1, 512], F32, tag="ir")
                nc.vector.reciprocal_approx_fast(out=ir[:, :cn],
                                                 in_=ss_ps[:, :cn])
                rr = wk.tile([1, 512], F16, tag="rr")
                nc.scalar.activation(out=rr[:, :cn], in_=ir[:, :cn],
                                     func=AFT.Sqrt, bias=0.0, scale=1.0)
                R_ps = pst.tile([128, 512], F32, tag="pp", bufs=3)
                nc.tensor.matmul(R_ps[:, :cn], ones_r, rr[:, :cn],
                                 start=True, stop=True)
                return R_ps

            def score(t0, nt):
                """U_sb[q, 4t:4(t+nt)] = qf[:,64t:...]^T sf[:,4t:...] x nt."""
                U_ps = pst.tile([64, 32], F32, tag="sc", bufs=1)
                for j in range(nt):
                    t = t0 + j
                    nc.tensor.matmul(U_ps[0:64, 4 * j:4 * j + 4],
                                     qf_t[:, 64 * t:64 * t + 64],
                                     sf_t[:, 4 * t:4 * t + 4],
                                     start=True, stop=True)
                nc.vector.tensor_copy(
                    out=U_sb[0:64, 4 * t0:4 * (t0 + nt)],
                    in_=U_ps[0:64, 0:4 * nt])

            def query_tail_fast(u_ps, qf_off, cn):
                """qf = normalize(u); u = Wu x accumulated during streaming."""
                ff = wk.tile([128, 512], F16, tag="ff")
                nc.vector.tensor_copy(out=ff[:, :cn], in_=u_ps[:, :cn])
                sq = wk.tile([128, 512], F16, tag="sq")
                nc.scalar.activation(out=sq[:, :cn], in_=u_ps[:, :cn],
                                     func=AFT.Square, bias=0.0, scale=1.0)
                RN = rstd(sq, cn, red_q)
                nc.vector.tensor_mul(out=qf_t[:, qf_off:qf_off + cn],
                                     in0=ff[:, :cn], in1=RN[:, :cn])
                score(qf_off // 64, cn // 64)

            def query_tail_gen(c_ps, qf_off, cn):
                cf = wk.tile([128, 512], F16, tag="cf")
                nc.vector.tensor_scalar_add(out=cf[:, :cn], in0=c_ps[:, :cn],
                                            scalar1=bc_a)
                sq = wk.tile([128, 512], F16, tag="sq")
                nc.scalar.activation(out=sq[:, :cn], in_=c_ps[:, :cn],
                                     func=AFT.Square, bias=bc_a, scale=1.0)
                R1 = rstd(sq, cn, red_ln)
                h = wk.tile([128, 512], F16, tag="h")
                nc.vector.tensor_mul(out=h[:, :cn], in0=cf[:, :cn],
                                     in1=R1[:, :cn])
                v_ps = pst.tile([128, 512], F32, tag="pp", bufs=3)
                nc.tensor.matmul(v_ps[:, :cn], wv_t, h[:, :cn],
                                 start=True, stop=True)
                vf = wk.tile([128, 512], F16, tag="vf")
                nc.vector.tensor_scalar_add(out=vf[:, :cn], in0=v_ps[:, :cn],
                                            scalar1=bv_a)
                sq2 = wk.tile([128, 512], F16, tag="sq")
                nc.scalar.activation(out=sq2[:, :cn], in_=v_ps[:, :cn],
                                     func=AFT.Square, bias=bv_a, scale=1.0)
                R2 = rstd(sq2, cn, red_ln)
                z = wk.tile([128, 512], F16, tag="h")
                nc.vector.tensor_mul(out=z[:, :cn], in0=vf[:, :cn],
                                     in1=R2[:, :cn])
                o_ps = pst.tile([128, 512], F32, tag="pp", bufs=3)
                nc.tensor.matmul(o_ps[:, :cn], wo_t, z[:, :cn],
                                 start=True, stop=True)
                ff = wk.tile([128, 512], F16, tag="ff")
                nc.vector.tensor_scalar_add(out=ff[:, :cn], in0=o_ps[:, :cn],
                                            scalar1=bo_a)
                sq3 = wk.tile([128, 512], F16, tag="sq")
                nc.scalar.activation(out=sq3[:, :cn], in_=o_ps[:, :cn],
                                     func=AFT.Square, bias=bo_a, scale=1.0)
                RN = rstd(sq3, cn, red_q)
                nc.vector.tensor_mul(out=qf_t[:, qf_off:qf_off + cn],
                                     in0=ff[:, :cn], in1=RN[:, :cn])
                score(qf_off // 64, cn // 64)

            query_tail = query_tail_fast if fast else query_tail_gen

            def support_tail(cS_ps):
                cfS = wk.tile([128, SROWS], F16, tag="cfS")
                if fast:
                    nc.vector.tensor_copy(out=cfS, in_=cS_ps[:, 0:SROWS])
                else:
                    nc.vector.tensor_scalar_add(out=cfS, in0=cS_ps[:, 0:SROWS],
                                                scalar1=bc_a)
                sqS = wk.tile([128, SROWS], F16, tag="sqS")
                if fast:
                    nc.scalar.activation(out=sqS, in_=cS_ps[:, 0:SROWS],
                                         func=AFT.Square, bias=0.0, scale=1.0)
                else:
                    nc.scalar.activation(out=sqS, in_=cS_ps[:, 0:SROWS],
                                         func=AFT.Square, bias=bc_a, scale=1.0)
                R1 = rstd(sqS, SROWS, red_ln)
                hs = wk.tile([128, SROWS], F16, tag="hs")
                nc.vector.tensor_mul(out=hs, in0=cfS, in1=R1[:, :SROWS])
                q_ps = pst.tile([128, 512], F32, tag="pp", bufs=3)
                nc.tensor.matmul(q_ps[:, :SROWS], wq_t, hs, start=True, stop=True)
                qTb = wk.tile([128, SROWS], F16, tag="qTb")
                if fast:
                    nc.vector.tensor_copy(out=qTb, in_=q_ps[:, :SROWS])
                else:
                    nc.vector.tensor_scalar_add(out=qTb, in0=q_ps[:, :SROWS],
                                                scalar1=bq_a)
                k_ps = pst.tile([128, 512], F32, tag="pp", bufs=3)
                nc.tensor.matmul(k_ps[:, :SROWS], wk_t, hs, start=True, stop=True)
                kTb = wk.tile([128, SROWS], F16, tag="kTb")
                if fast:
                    nc.vector.tensor_copy(out=kTb, in_=k_ps[:, :SROWS])
                else:
                    nc.vector.tensor_scalar_add(out=kTb, in0=k_ps[:, :SROWS],
                                                scalar1=bk_a)
                # vn = (Wv h)^T (+ 1 (x) bv in the general path) : [srows, hid]
                vn_ps = pst.tile([128, 512], F32, tag="pp", bufs=3)
                nc.tensor.matmul(vn_ps[:, :HID], hs, wv_t, start=True,
                                 stop=fast)
                if not fast:
                    nc.tensor.matmul(vn_ps[:, :HID], ones_row[:, 0:SROWS],
                                     bvrow_t, start=False, stop=True)
                vn_sb = wk.tile([SROWS, HID], F16, tag="vn")
                nc.vector.tensor_copy(out=vn_sb, in_=vn_ps[:, :HID])
                # sT[k_row, q_row] = k^T q ; unnormalized masked exp
                sT_ps = pst.tile([128, 512], F32, tag="pp", bufs=3)
                nc.tensor.matmul(sT_ps[:, :SROWS], kTb, qTb, start=True,
                                 stop=True)
                aT = wk.tile([SROWS, SROWS], F16, tag="aT")
                nc.scalar.activation(out=aT, in_=sT_ps[:, :SROWS], func=AFT.Exp,
                                     bias=0.0, scale=1.0)
                am = wk.tile([SROWS, SROWS], F16, tag="am")
                nc.vector.tensor_mul(out=am, in0=aT, in1=mask_t)
                ctx_ps = pst.tile([128, 512], F32, tag="pp", bufs=3)
                nc.tensor.matmul(ctx_ps[:, :SROWS], vn_sb, am, start=True,
                                 stop=True)
                cfx = wk.tile([128, SROWS], F16, tag="cfS")
                nc.vector.tensor_copy(out=cfx, in_=ctx_ps[:, :SROWS])
                if fast:
                    zx = cfx           # LN2 rstd cancels in the final normalize
                else:
                    sqx = wk.tile([128, SROWS], F16, tag="sqS")
                    nc.scalar.activation(out=sqx, in_=ctx_ps[:, :SROWS],
                                         func=AFT.Square, bias=0.0, scale=1.0)
                    R2 = rstd(sqx, SROWS, red_ln)
                    zx = wk.tile([128, SROWS], F16, tag="hs")
                    nc.vector.tensor_mul(out=zx, in0=cfx, in1=R2[:, :SROWS])
                o_ps = pst.tile([128, 512], F32, tag="pp", bufs=3)
                nc.tensor.matmul(o_ps[:, :SROWS], wo_t, zx, start=True, stop=True)
                ffx = wk.tile([128, SROWS], F16, tag="ffS")
                if fast:
                    nc.vector.tensor_copy(out=ffx, in_=o_ps[:, :SROWS])
                else:
                    nc.vector.tensor_scalar_add(out=ffx, in0=o_ps[:, :SROWS],
                                                scalar1=bo_a)
                sqf = wk.tile([128, SROWS], F16, tag="sqS")
                if fast:
                    nc.scalar.activation(out=sqf, in_=o_ps[:, :SROWS],
                                         func=AFT.Square, bias=0.0, scale=1.0)
                else:
                    nc.scalar.activation(out=sqf, in_=o_ps[:, :SROWS],
                                         func=AFT.Square, bias=bo_a, scale=1.0)
                RN = rstd(sqf, SROWS, red_s)   # = 10/||f||
                nc.vector.tensor_mul(out=sf_t[:], in0=ffx, in1=RN[:, :SROWS])

            def qw_k(k):
                return (wcwu_t[:, k, 128:256] if fast
                        else wcwu_t[:, k, 0:128])

            def out_dma(t0, nt):
                dst = _b.AP(tensor=out_d[:].tensor,
                            offset=out_d[:].offset + 256 * t0,
                            ap=[[4, 64], [256, nt], [1, 4]])
                nc.sync.dma_start(
                    out=dst,
                    in_=U_sb[0:64, 4 * t0:4 * (t0 + nt)].rearrange(
                        "p (g b) -> p g b", b=4))

            # ---- group 1: support + q0 ----
            with tc.tile_pool(name="psA", bufs=1, space="PSUM") as psA:
                cS_ps = psA.tile([128, SROWS], F32)
                u0_ps = psA.tile([128, 512], F32)
                for k in range(KT):
                    nc.tensor.matmul(cS_ps[:], wcwu_t[:, k, 0:128],
                                     xg1_t[:, k, 0:128],
                                     start=(k == 0), stop=(k == KT - 1))
                    nc.tensor.matmul(u0_ps[:], qw_k(k),
                                     xg1_t[:, k, 128:640],
                                     start=(k == 0), stop=(k == KT - 1))
                support_tail(cS_ps)
                query_tail(u0_ps, 0, 512)

            # ---- group 2: q1 + q2 ----
            with tc.tile_pool(name="psB", bufs=1, space="PSUM") as psB:
                u1_ps = psB.tile([128, 512], F32)
                u2_ps = psB.tile([128, 512], F32)
                for k in range(KT):
                    nc.tensor.matmul(u1_ps[:], qw_k(k),
                                     xg2_t[:, k, 0:512],
                                     start=(k == 0), stop=(k == KT - 1))
                    nc.tensor.matmul(u2_ps[:], qw_k(k),
                                     xg2_t[:, k, 512:1024],
                                     start=(k == 0), stop=(k == KT - 1))
                query_tail(u1_ps, 512, 512)
                query_tail(u2_ps, 1024, 512)
                out_dma(0, 24)

            # ---- group 3: q3 in two 256-col halves ----
            with tc.tile_pool(name="psC", bufs=1, space="PSUM") as psC:
                ua_ps = psC.tile([128, 320], F32)
                ub_ps = psC.tile([128, 192], F32)
                for k in range(KT):
                    nc.tensor.matmul(ua_ps[:], qw_k(k),
                                     xg3a_t[:, k, :],
                                     start=(k == 0), stop=(k == KT - 1))
                for k in range(KT):
                    nc.tensor.matmul(ub_ps[:], qw_k(k),
                                     xg3b_t[:, k, :],
                                     start=(k == 0), stop=(k == KT - 1))
                query_tail(ua_ps, 1536, 320)
                out_dma(24, 5)
                query_tail(ub_ps, 1856, 192)
                out_dma(29, 3)

    lp.__exit__(None, None, None)
    nc.compile()
    return nc


def _host_prep(inputs):
    f32, f16 = np.float32, np.float16
    Wi, Wt = np.asarray(inputs["Wi"], f32), np.asarray(inputs["Wt"], f32)
    bi, bt = np.asarray(inputs["bi"], f32), np.asarray(inputs["bt"], f32)
    g1, b1 = np.asarray(inputs["g1"], f32), np.asarray(inputs["b1"], f32)
    g2, b2 = np.asarray(inputs["g2"], f32), np.asarray(inputs["b2"], f32)
    Wq, bq = np.asarray(inputs["Wq"], f32), np.asarray(inputs["bq"], f32)
    Wk, bk = np.asarray(inputs["Wk"], f32), np.asarray(inputs["bk"], f32)
    Wv, bv = np.asarray(inputs["Wv"], f32), np.asarray(inputs["bv"], f32)
    Wo, bo = np.asarray(inputs["Wo"], f32), np.asarray(inputs["bo"], f32)

    Wc = np.concatenate([Wi, Wt], axis=1)          # [128, 2816]
    bc = bi + bt
    Wc_c = Wc - Wc.mean(axis=0, keepdims=True)     # fold LN1 mean
    bc_c = bc - bc.mean()

    Wq_f = (Wq * g1[None, :]) * SCALE_INV
    bq_f = (bq + Wq @ b1) * SCALE_INV
    Wk_f = Wk * g1[None, :]
    bk_f = bk + Wk @ b1
    Wv_f = Wv * g1[None, :]
    bv_f = bv + Wv @ b1
    Wv_c = Wv_f - Wv_f.mean(axis=0, keepdims=True)  # fold LN2 mean
    bv_c = bv_f - bv_f.mean()
    Wo_f = Wo * g2[None, :]
    bo_f = bo + Wo @ b2
    Wov = Wo_f @ Wv_c                               # combined v+o projection
    Wu = Wov @ Wc_c                                 # full query-path fold

    fast = all(np.abs(b).max() < 1e-12
               for b in (bc_c, bv_c, bo_f, bq_f, bk_f))

    blk = np.arange(SROWS) // S
    mask01 = (blk[:, None] == blk[None, :]).astype(f16)
    consts = np.zeros((128, 772), f16)
    consts[:, 0:640] = np.concatenate(
        [Wq_f.T, Wk_f.T, Wv_c.T, Wo_f.T, Wov.T], axis=1).astype(f16)
    consts[:, 640] = f16(1.0 / HID)
    consts[:, 641] = f16(1.0)
    consts[:, 642] = f16(0.01)
    consts[:, 644:772] = mask01
    row0 = np.zeros((1, 768), f16)
    row0[0, 0:640] = 1.0
    row0[0, 640:768] = bv_c.astype(f16)

    def pack_kmajor(a):   # [feat, cols] -> [128, KT*cols] (p, k, c)
        cols = a.shape[1]
        return np.ascontiguousarray(
            a.reshape(KT, 128, cols).transpose(1, 0, 2).reshape(128, -1)
        )

    wcwu = np.concatenate([Wc_c.T.astype(f16).reshape(KT, 128, 128),
                           Wu.T.astype(f16).reshape(KT, 128, 128)],
                          axis=2)          # [KT, 128, 256] (wc_k | wu_k)
    common = {
        "wcwu": np.ascontiguousarray(
            wcwu.transpose(1, 0, 2).reshape(128, -1)),
        "consts": consts,
        "row0": row0,
    }
    if not fast:
        common["biases"] = np.ascontiguousarray(
            np.stack([bc_c, bq_f, bk_f, bv_c, bo_f], axis=1))

    si = np.asarray(inputs["support_images"], f32)
    st = np.asarray(inputs["support_texts"], f32)
    qi = np.asarray(inputs["query_images"], f32)
    qt = np.asarray(inputs["query_texts"], f32)

    in_maps = []
    for m in range(NCORES):
        ts = slice(m * TPC, (m + 1) * TPC)
        Xq = np.concatenate([qi[ts].reshape(QROWS, DI),
                             qt[ts].reshape(QROWS, DTXT)], axis=1)
        Xs = np.concatenate([si[ts].reshape(SROWS, DI),
                             st[ts].reshape(SROWS, DTXT)], axis=1)
        xT = np.concatenate([Xs, Xq], axis=0).T.astype(f16)  # [2816, 2176]
        in_maps.append({
            "x1": pack_kmajor(xT[:, 0:640]),
            "x2": pack_kmajor(xT[:, 640:1664]),
            "x3a": pack_kmajor(xT[:, 1664:1984]),
            "x3b": pack_kmajor(xT[:, 1984:2176]),
            **common,
        })
    return in_maps, fast


def _run(in_maps, fast, trace=False, **kw):
    from concourse.bass_utils import run_bass_kernel_spmd
    if fast not in _progs:
        _progs[fast] = _build(fast)
    return run_bass_kernel_spmd(_progs[fast], in_maps, list(range(NCORES)),
                                trace=trace, **kw)


def kernel(**inputs) -> np.ndarray:
    in_maps, fast = _host_prep(inputs)
    res = _run(in_maps, fast)
    return np.concatenate([res.results[m]["logits"] for m in range(NCORES)],
                          axis=0)


# revision 14
# speedup vs baseline: 2.9946x; 1.0030x over previous
"""Trainium2 Bass kernel for nn_MetaLearner (meta-learning attention + cosine
prototype scoring), data-parallel over tasks on 8 NeuronCores.

Math (per task):
  c   = [img, txt] @ Wc.T + bc                (Wc = concat(Wi, Wt))
  h   = LN1(c);  q,k,v = h @ W{q,k,v}.T + b   (queries: seqlen=1 -> ctx = v)
  ctx = softmax(q k^T / sqrt(128)) v          (support: seqlen=4)
  f   = LN2(ctx) @ Wo.T + bo
  logits[t,q,c] = 10 * cos(qf[t,q], sf[t,c])

Key tricks:
  - LN gains/biases and mean-subtractions folded into weights on host.
  - Softmax denominator (and max-subtraction) dropped: LN2's rstd cancels
    any positive per-column scale of ctx; mean-centering folded into Wv.
  - When bc/bv/bo fold to zero (true for the reference initialization),
    every per-column scale cancels through the final cosine normalize, so
    the whole query path collapses to qf = normalize(Wu x) with
    Wu = (Wo Wv_c) Wc_c folded on host -- queries never materialize c, h,
    v, or f; the projection accumulates directly during streaming.
    Support keeps LN1 (softmax is scale-sensitive) but skips LN2's rstd.
    A general fallback path keeps the full math.
  - rstd = Sqrt(reciprocal_approx_fast(scale * colsum(x^2))): the scale
    rides in the reduction weights, DVE does the reciprocal, ACT only ever
    evaluates Sqrt (plus one Exp for softmax) -> ~2 table switches total.
  - No transposes: attention scores computed pre-transposed (sT = kT^T qT)
    and v computed pre-transposed (vn = h^T WvT) by operand swapping.
  - All PE operands fp16 (1 cyc/row); f32 accumulation in PSUM.
  - Inputs streamed as fp16, host-packed per column-group so every stream
    DMA is one contiguous 1.4-5.6 MB transfer. The last group is only 256
    columns so the final (unhidable) tail chain is short.
On-chip layout is "transposed" throughout: activations are [hid, rows].
"""
import sys
sys.path.insert(0, "/opt/trn_rl_repo")
import numpy as np

HID = 128
T, Q, S = 256, 64, 4
DI, DTXT = 2048, 768
NCORES = 8
TPC = T // NCORES               # 32 tasks per core
FEAT = DI + DTXT                # 2816
KT = FEAT // 128                # 22 contraction chunks
QROWS = TPC * Q                 # 2048 query rows per core
SROWS = TPC * S                 # 128 support rows per core
ROWS = QROWS + SROWS            # 2176
SCALE_INV = 1.0 / (np.sqrt(HID) + 1e-8)

_progs = {}  # cached compiled Bass programs, keyed by fast-path flag


def _build(fast):
    import concourse.bacc as bacc
    import concourse.tile as tile
    import concourse.mybir as mybir
    import concourse.bass as _b

    F32 = mybir.dt.float32
    F16 = mybir.dt.float16
    AFT = mybir.ActivationFunctionType

    nc = bacc.Bacc()
    x1_d = nc.declare_dram_parameter("x1", [128, KT * 640], F16, isOutput=False)
    x2_d = nc.declare_dram_parameter("x2", [128, KT * 1024], F16, isOutput=False)
    x3a_d = nc.declare_dram_parameter("x3a", [128, KT * 320], F16,
                                      isOutput=False)
    x3b_d = nc.declare_dram_parameter("x3b", [128, KT * 192], F16,
                                      isOutput=False)
    wcwu_d = nc.declare_dram_parameter("wcwu", [128, KT * 256], F16,
                                       isOutput=False)
    cst_d = nc.declare_dram_parameter("consts", [128, 772], F16, isOutput=False)
    row0_d = nc.declare_dram_parameter("row0", [1, 768], F16, isOutput=False)
    bias_d = (None if fast else
              nc.declare_dram_parameter("biases", [HID, 5], F32,
                                        isOutput=False))
    out_d = nc.declare_dram_parameter("logits", [TPC, Q, S], F32, isOutput=True)

    lp = nc.allow_low_precision(reason="fp16 streaming with f32 accumulation")
    lp.__enter__()

    with tile.TileContext(nc) as tc:
        with (
            tc.tile_pool(name="wts", bufs=1) as wts,
            tc.tile_pool(name="xg1p", bufs=1) as xg1p,
            tc.tile_pool(name="xg2p", bufs=1) as xg2p,
            tc.tile_pool(name="xg3p", bufs=1) as xg3p,
            tc.tile_pool(name="qfp", bufs=1) as qfp,
            tc.tile_pool(name="wk", bufs=3) as wk,
            tc.tile_pool(name="pst", bufs=1, space="PSUM") as pst,
        ):
            # ---- loads, in stream order ----
            wcwu_t = wts.tile([128, KT, 256], F16)
            nc.sync.dma_start(out=wcwu_t, in_=wcwu_d[:])
            xg1_t = xg1p.tile([128, KT, 640], F16)
            nc.sync.dma_start(out=xg1_t[:, 0:11, :], in_=x1_d[:, 0:11 * 640])
            nc.sync.dma_start(out=xg1_t[:, 11:KT, :], in_=x1_d[:, 11 * 640:])
            cst_t = wts.tile([128, 772], F16)
            nc.sync.dma_start(out=cst_t, in_=cst_d[:])
            wq_t = cst_t[:, 0 * HID:1 * HID]
            wk_t = cst_t[:, 1 * HID:2 * HID]
            wv_t = cst_t[:, 2 * HID:3 * HID]
            wo_t = cst_t[:, 3 * HID:4 * HID]
            wov_t = cst_t[:, 4 * HID:5 * HID]
            red_ln = cst_t[:, 640:641]           # 1/128
            red_q = cst_t[:, 641:642]            # 1.0
            red_s = cst_t[:, 642:643]            # 0.01
            mask_t = cst_t[:, 644:772]           # [128, 128] 0/1 block mask
            row0_t = wts.tile([1, 768], F16)
            nc.sync.dma_start(out=row0_t, in_=row0_d[:])
            ones_r = row0_t[:, 0:128]            # lhsT [K=1, M=128] for bcast
            ones_row = row0_t[:, 0:640]          # all-ones rhs
            bvrow_t = row0_t[:, 640:768]
            if not fast:
                bias_t = wts.tile([HID, 5], F32)
                nc.sync.dma_start(out=bias_t, in_=bias_d[:])
                bc_a, bq_a, bk_a, bv_a, bo_a = (
                    bias_t[:, i:i + 1] for i in range(5))
            else:
                bc_a = bq_a = bk_a = bv_a = bo_a = None
            xg2_t = xg2p.tile([128, KT, 1024], F16)
            for (a, b) in ((0, 6), (6, 11), (11, 17), (17, KT)):
                nc.sync.dma_start(out=xg2_t[:, a:b, :],
                                  in_=x2_d[:, a * 1024:b * 1024])
            xg3a_t = xg3p.tile([128, KT, 320], F16)
            nc.sync.dma_start(out=xg3a_t[:, 0:11, :], in_=x3a_d[:, 0:11 * 320])
            nc.sync.dma_start(out=xg3a_t[:, 11:KT, :], in_=x3a_d[:, 11 * 320:])
            xg3b_t = xg3p.tile([128, KT, 192], F16)
            nc.sync.dma_start(out=xg3b_t[:, 0:11, :], in_=x3b_d[:, 0:11 * 192])
            nc.sync.dma_start(out=xg3b_t[:, 11:KT, :], in_=x3b_d[:, 11 * 192:])

            qf_t = qfp.tile([128, QROWS], F16)
            sf_t = qfp.tile([128, SROWS], F16)
            U_sb = qfp.tile([64, 2 * Q], F32)

            def rstd(sq_sb, cn, red):
                """1/sqrt(red . sq) broadcast to [128, cn] PSUM (f16 path)."""
                ss_ps = pst.tile([1, 512], F32, tag="ss", bufs=2)
                nc.tensor.matmul(ss_ps[:, :cn], red, sq_sb[:, :cn],
                                 start=True, stop=True)
                ir = wk.tile([1, 512], F32, tag="ir")
                nc.vector.reciprocal_approx_fast(out=ir[:, :cn],
                                                 in_=ss_ps[:, :cn])
                rr = wk.tile([1, 512], F16, tag="rr")
                nc.scalar.activation(out=rr[:, :cn], in_=ir[:, :cn],
                                     func=AFT.Sqrt, bias=0.0, scale=1.0)
                R_ps = pst.tile([128, 512], F32, tag="pp", bufs=3)
                nc.tensor.matmul(R_ps[:, :cn], ones_r, rr[:, :cn],
                                 start=True, stop=True)
                return R_ps

            def score(t0, nt):
                """U_sb[q, 4t:4(t+nt)] = qf[:,64t:...]^T sf[:,4t:...] x nt."""
                U_ps = pst.tile([64, 32], F32, tag="sc", bufs=1)
                for j in range(nt):
                    t = t0 + j
                    nc.tensor.matmul(U_ps[0:64, 4 * j:4 * j + 4],
                                     qf_t[:, 64 * t:64 * t + 64],
                                     sf_t[:, 4 * t:4 * t + 4],
                                     start=True, stop=True)
                nc.vector.tensor_copy(
                    out=U_sb[0:64, 4 * t0:4 * (t0 + nt)],
                    in_=U_ps[0:64, 0:4 * nt])

            def query_tail_fast(u_ps, qf_off, cn):
                """qf = normalize(u); u = Wu x accumulated during streaming."""
                ff = wk.tile([128, 512], F16, tag="ff")
                nc.vector.tensor_copy(out=ff[:, :cn], in_=u_ps[:, :cn])
                sq = wk.tile([128, 512], F16, tag="sq")
                nc.scalar.activation(out=sq[:, :cn], in_=u_ps[:, :cn],
                                     func=AFT.Square, bias=0.0, scale=1.0)
                RN = rstd(sq, cn, red_q)
                nc.vector.tensor_mul(out=qf_t[:, qf_off:qf_off + cn],
                                     in0=ff[:, :cn], in1=RN[:, :cn])
                score(qf_off // 64, cn // 64)

            def query_tail_gen(c_ps, qf_off, cn):
                cf = wk.tile([128, 512], F16, tag="cf")
                nc.vector.tensor_scalar_add(out=cf[:, :cn], in0=c_ps[:, :cn],
                                            scalar1=bc_a)
                sq = wk.tile([128, 512], F16, tag="sq")
                nc.scalar.activation(out=sq[:, :cn], in_=c_ps[:, :cn],
                                     func=AFT.Square, bias=bc_a, scale=1.0)
                R1 = rstd(sq, cn, red_ln)
                h = wk.tile([128, 512], F16, tag="h")
                nc.vector.tensor_mul(out=h[:, :cn], in0=cf[:, :cn],
                                     in1=R1[:, :cn])
                v_ps = pst.tile([128, 512], F32, tag="pp", bufs=3)
                nc.tensor.matmul(v_ps[:, :cn], wv_t, h[:, :cn],
                                 start=True, stop=True)
                vf = wk.tile([128, 512], F16, tag="vf")
                nc.vector.tensor_scalar_add(out=vf[:, :cn], in0=v_ps[:, :cn],
                                            scalar1=bv_a)
                sq2 = wk.tile([128, 512], F16, tag="sq")
                nc.scalar.activation(out=sq2[:, :cn], in_=v_ps[:, :cn],
                                     func=AFT.Square, bias=bv_a, scale=1.0)
                R2 = rstd(sq2, cn, red_ln)
                z = wk.tile([128, 512], F16, tag="h")
                nc.vector.tensor_mul(out=z[:, :cn], in0=vf[:, :cn],
                                     in1=R2[:, :cn])
                o_ps = pst.tile([128, 512], F32, tag="pp", bufs=3)
                nc.tensor.matmul(o_ps[:, :cn], wo_t, z[:, :cn],
                                 start=True, stop=True)
                ff = wk.tile([128, 512], F16, tag="ff")
                nc.vector.tensor_scalar_add(out=ff[:, :cn], in0=o_ps[:, :cn],
                                            scalar1=bo_a)
                sq3 = wk.tile([128, 512], F16, tag="sq")
                nc.scalar.activation(out=sq3[:, :cn], in_=o_ps[:, :cn],
                                     func=AFT.Square, bias=bo_a, scale=1.0)
                RN = rstd(sq3, cn, red_q)
                nc.vector.tensor_mul(out=qf_t[:, qf_off:qf_off + cn],
                                     in0=ff[:, :cn], in1=RN[:, :cn])
                score(qf_off // 64, cn // 64)

            query_tail = query_tail_fast if fast else query_tail_gen

            def support_tail(cS_ps):
                cfS = wk.tile([128, SROWS], F16, tag="cfS")
                if fast:
                    nc.vector.tensor_copy(out=cfS, in_=cS_ps[:, 0:SROWS])
                else:
                    nc.vector.tensor_scalar_add(out=cfS, in0=cS_ps[:, 0:SROWS],
                                                scalar1=bc_a)
                sqS = wk.tile([128, SROWS], F16, tag="sqS")
                if fast:
                    nc.scalar.activation(out=sqS, in_=cS_ps[:, 0:SROWS],
                                         func=AFT.Square, bias=0.0, scale=1.0)
                else:
                    nc.scalar.activation(out=sqS, in_=cS_ps[:, 0:SROWS],
                                         func=AFT.Square, bias=bc_a, scale=1.0)
                R1 = rstd(sqS, SROWS, red_ln)
                hs = wk.tile([128, SROWS], F16, tag="hs")
                nc.vector.tensor_mul(out=hs, in0=cfS, in1=R1[:, :SROWS])
                q_ps = pst.tile([128, 512], F32, tag="pp", bufs=3)
                nc.tensor.matmul(q_ps[:, :SROWS], wq_t, hs, start=True, stop=True)
                qTb = wk.tile([128, SROWS], F16, tag="qTb")
                if fast:
                    nc.vector.tensor_copy(out=qTb, in_=q_ps[:, :SROWS])
                else:
                    nc.vector.tensor_scalar_add(out=qTb, in0=q_ps[:, :SROWS],
                                                scalar1=bq_a)
                k_ps = pst.tile([128, 512], F32, tag="pp", bufs=3)
                nc.tensor.matmul(k_ps[:, :SROWS], wk_t, hs, start=True, stop=True)
                kTb = wk.tile([128, SROWS], F16, tag="kTb")
                if fast:
                    nc.vector.tensor_copy(out=kTb, in_=k_ps[:, :SROWS])
                else:
                    nc.vector.tensor_scalar_add(out=kTb, in0=k_ps[:, :SROWS],
                                                scalar1=bk_a)
                # vn = (Wv h)^T (+ 1 (x) bv in the general path) : [srows, hid]
                vn_ps = pst.tile([128, 512], F32, tag="pp", bufs=3)
                nc.tensor.matmul(vn_ps[:, :HID], hs, wv_t, start=True,
                                 stop=fast)
                if not fast:
                    nc.tensor.matmul(vn_ps[:, :HID], ones_row[:, 0:SROWS],
                                     bvrow_t, start=False, stop=True)
                vn_sb = wk.tile([SROWS, HID], F16, tag="vn")
                nc.vector.tensor_copy(out=vn_sb, in_=vn_ps[:, :HID])
                # sT[k_row, q_row] = k^T q ; unnormalized masked exp
                sT_ps = pst.tile([128, 512], F32, tag="pp", bufs=3)
                nc.tensor.matmul(sT_ps[:, :SROWS], kTb, qTb, start=True,
                                 stop=True)
                aT = wk.tile([SROWS, SROWS], F16, tag="aT")
                nc.scalar.activation(out=aT, in_=sT_ps[:, :SROWS], func=AFT.Exp,
                                     bias=0.0, scale=1.0)
                am = wk.tile([SROWS, SROWS], F16, tag="am")
                nc.vector.tensor_mul(out=am, in0=aT, in1=mask_t)
                ctx_ps = pst.tile([128, 512], F32, tag="pp", bufs=3)
                nc.tensor.matmul(ctx_ps[:, :SROWS], vn_sb, am, start=True,
                                 stop=True)
                cfx = wk.tile([128, SROWS], F16, tag="cfS")
                nc.vector.tensor_copy(out=cfx, in_=ctx_ps[:, :SROWS])
                if fast:
                    zx = cfx           # LN2 rstd cancels in the final normalize
                else:
                    sqx = wk.tile([128, SROWS], F16, tag="sqS")
                    nc.scalar.activation(out=sqx, in_=ctx_ps[:, :SROWS],
                                         func=AFT.Square, bias=0.0, scale=1.0)
                    R2 = rstd(sqx, SROWS, red_ln)
                    zx = wk.tile([128, SROWS], F16, tag="hs")
                    nc.vector.tensor_mul(out=zx, in0=cfx, in1=R2[:, :SROWS])
                o_ps = pst.tile([128, 512], F32, tag="pp", bufs=3)
                nc.tensor.matmul(o_ps[:, :SROWS], wo_t, zx, start=True, stop=True)
                ffx = wk.tile([128, SROWS], F16, tag="ffS")
                if fast:
                    nc.vector.tensor_copy(out=ffx, in_=o_ps[:, :SROWS])
                else:
                    nc.vector.tensor_scalar_add(out=ffx, in0=o_ps[:, :SROWS],
                                                scalar1=bo_a)
                sqf = wk.tile([128, SROWS], F16, tag="sqS")
                if fast:
                    nc.scalar.activation(out=sqf, in_=o_ps[:, :SROWS],
                                         func=AFT.Square, bias=0.0, scale=1.0)
                else:
                    nc.scalar.activation(out=sqf, in_=o_ps[:, :SROWS],
                                         func=AFT.Square, bias=bo_a, scale=1.0)
                RN = rstd(sqf, SROWS, red_s)   # = 10/||f||
                nc.vector.tensor_mul(out=sf_t[:], in0=ffx, in1=RN[:, :SROWS])

            def qw_k(k):
                return (wcwu_t[:, k, 128:256] if fast
                        else wcwu_t[:, k, 0:128])

            def out_dma(t0, nt):
                dst = _b.AP(tensor=out_d[:].tensor,
                            offset=out_d[:].offset + 256 * t0,
                            ap=[[4, 64], [256, nt], [1, 4]])
                nc.sync.dma_start(
                    out=dst,
                    in_=U_sb[0:64, 4 * t0:4 * (t0 + nt)].rearrange(
                        "p (g b) -> p g b", b=4))

            # ---- group 1: support + q0 ----
            with tc.tile_pool(name="psA", bufs=1, space="PSUM") as psA:
                cS_ps = psA.tile([128, SROWS], F32)
                u0_ps = psA.tile([128, 512], F32)
                for k in range(KT):
                    nc.tensor.matmul(cS_ps[:], wcwu_t[:, k, 0:128],
                                     xg1_t[:, k, 0:128],
                                     start=(k == 0), stop=(k == KT - 1))
                    nc.tensor.matmul(u0_ps[:], qw_k(k),
                                     xg1_t[:, k, 128:640],
                                     start=(k == 0), stop=(k == KT - 1))
                support_tail(cS_ps)
                query_tail(u0_ps, 0, 512)

            # ---- group 2: q1 + q2 ----
            with tc.tile_pool(name="psB", bufs=1, space="PSUM") as psB:
                u1_ps = psB.tile([128, 512], F32)
                u2_ps = psB.tile([128, 512], F32)
                for k in range(KT):
                    nc.tensor.matmul(u1_ps[:], qw_k(k),
                                     xg2_t[:, k, 0:512],
                                     start=(k == 0), stop=(k == KT - 1))
                    nc.tensor.matmul(u2_ps[:], qw_k(k),
                                     xg2_t[:, k, 512:1024],
                                     start=(k == 0), stop=(k == KT - 1))
                query_tail(u1_ps, 512, 512)
                query_tail(u2_ps, 1024, 512)
                out_dma(0, 24)

            # ---- group 3: q3 in two 256-col halves ----
            with tc.tile_pool(name="psC", bufs=1, space="PSUM") as psC:
                ua_ps = psC.tile([128, 320], F32)
                ub_ps = psC.tile([128, 192], F32)
                for k in range(KT):
                    nc.tensor.matmul(ua_ps[:], qw_k(k),
                                     xg3a_t[:, k, :],
                                     start=(k == 0), stop=(k == KT - 1))
                for k in range(KT):
                    nc.tensor.matmul(ub_ps[:], qw_k(k),
                                     xg3b_t[:, k, :],
                                     start=(k == 0), stop=(k == KT - 1))
                query_tail(ua_ps, 1536, 320)
                query_tail(ub_ps, 1856, 192)
                out_dma(24, 8)

    lp.__exit__(None, None, None)
    nc.compile()
    return nc


def _host_prep(inputs):
    f32, f16 = np.float32, np.float16
    Wi, Wt = np.asarray(inputs["Wi"], f32), np.asarray(inputs["Wt"], f32)
    bi, bt = np.asarray(inputs["bi"], f32), np.asarray(inputs["bt"], f32)
    g1, b1 = np.asarray(inputs["g1"], f32), np.asarray(inputs["b1"], f32)
    g2, b2 = np.asarray(inputs["g2"], f32), np.asarray(inputs["b2"], f32)
    Wq, bq = np.asarray(inputs["Wq"], f32), np.asarray(inputs["bq"], f32)
    Wk, bk = np.asarray(inputs["Wk"], f32), np.asarray(inputs["bk"], f32)
    Wv, bv = np.asarray(inputs["Wv"], f32), np.asarray(inputs["bv"], f32)
    Wo, bo = np.asarray(inputs["Wo"], f32), np.asarray(inputs["bo"], f32)

    Wc = np.concatenate([Wi, Wt], axis=1)          # [128, 2816]
    bc = bi + bt
    Wc_c = Wc - Wc.mean(axis=0, keepdims=True)     # fold LN1 mean
    bc_c = bc - bc.mean()

    Wq_f = (Wq * g1[None, :]) * SCALE_INV
    bq_f = (bq + Wq @ b1) * SCALE_INV
    Wk_f = Wk * g1[None, :]
    bk_f = bk + Wk @ b1
    Wv_f = Wv * g1[None, :]
    bv_f = bv + Wv @ b1
    Wv_c = Wv_f - Wv_f.mean(axis=0, keepdims=True)  # fold LN2 mean
    bv_c = bv_f - bv_f.mean()
    Wo_f = Wo * g2[None, :]
    bo_f = bo + Wo @ b2
    Wov = Wo_f @ Wv_c                               # combined v+o projection
    Wu = Wov @ Wc_c                                 # full query-path fold

    fast = all(np.abs(b).max() < 1e-12
               for b in (bc_c, bv_c, bo_f, bq_f, bk_f))

    blk = np.arange(SROWS) // S
    mask01 = (blk[:, None] == blk[None, :]).astype(f16)
    consts = np.zeros((128, 772), f16)
    consts[:, 0:640] = np.concatenate(
        [Wq_f.T, Wk_f.T, Wv_c.T, Wo_f.T, Wov.T], axis=1).astype(f16)
    consts[:, 640] = f16(1.0 / HID)
    consts[:, 641] = f16(1.0)
    consts[:, 642] = f16(0.01)
    consts[:, 644:772] = mask01
    row0 = np.zeros((1, 768), f16)
    row0[0, 0:640] = 1.0
    row0[0, 640:768] = bv_c.astype(f16)

    def pack_kmajor(a):   # [feat, cols] -> [128, KT*cols] (p, k, c)
        cols = a.shape[1]
        return np.ascontiguousarray(
            a.reshape(KT, 128, cols).transpose(1, 0, 2).reshape(128, -1)
        )

    wcwu = np.concatenate([Wc_c.T.astype(f16).reshape(KT, 128, 128),
                           Wu.T.astype(f16).reshape(KT, 128, 128)],
                          axis=2)          # [KT, 128, 256] (wc_k | wu_k)
    common = {
        "wcwu": np.ascontiguousarray(
            wcwu.transpose(1, 0, 2).reshape(128, -1)),
        "consts": consts,
        "row0": row0,
    }
    if not fast:
        common["biases"] = np.ascontiguousarray(
            np.stack([bc_c, bq_f, bk_f, bv_c, bo_f], axis=1))

    si = np.asarray(inputs["support_images"], f32)
    st = np.asarray(inputs["support_texts"], f32)
    qi = np.asarray(inputs["query_images"], f32)
    qt = np.asarray(inputs["query_texts"], f32)

    in_maps = []
    for m in range(NCORES):
        ts = slice(m * TPC, (m + 1) * TPC)
        Xq = np.concatenate([qi[ts].reshape(QROWS, DI),
                             qt[ts].reshape(QROWS, DTXT)], axis=1)
        Xs = np.concatenate([si[ts].reshape(SROWS, DI),
                             st[ts].reshape(SROWS, DTXT)], axis=1)
        xT = np.concatenate([Xs, Xq], axis=0).T.astype(f16)  # [2816, 2176]
        in_maps.append({
            "x1": pack_kmajor(xT[:, 0:640]),
            "x2": pack_kmajor(xT[:, 640:1664]),
            "x3a": pack_kmajor(xT[:, 1664:1984]),
            "x3b": pack_kmajor(xT[:, 1984:2176]),
            **common,
        })
    return in_maps, fast


def _run(in_maps, fast, trace=False, **kw):
    from concourse.bass_utils import run_bass_kernel_spmd
    if fast not in _progs:
        _progs[fast] = _build(fast)
    return run_bass_kernel_spmd(_progs[fast], in_maps, list(range(NCORES)),
                                trace=trace, **kw)


def kernel(**inputs) -> np.ndarray:
    in_maps, fast = _host_prep(inputs)
    res = _run(in_maps, fast)
    return np.concatenate([res.results[m]["logits"] for m in range(NCORES)],
                          axis=0)
